# revision 2
# baseline (speedup 1.0000x reference)
import math
from contextlib import ExitStack

import numpy as np

import concourse.bass as bass
import concourse.tile as tile
from concourse import bacc, mybir
from concourse.masks import make_identity

F32 = mybir.dt.float32
BF16 = mybir.dt.bfloat16
FP8 = mybir.dt.float8e4
AL = mybir.AluOpType
AF = mybir.ActivationFunctionType
DR = mybir.MatmulPerfMode.DoubleRow

C = 192          # channels
HEADS = 4
CH = C // HEADS  # 48
W = 128          # image width
SR = 16          # rows per stripe
PW = W + 2       # padded width (bf16 kvp tiles)
PWQ = 144        # padded width for fp8 q staging (16B-aligned pair strides)
PR = SR + 2      # padded rows per stripe

S_WEFF = 256.0   # fp8 scale for q-conv weights (cancels in l2-norm)
S_KV = 64.0      # (unused: kv conv stays bf16 -- v cannot tolerate fp8)

# tap index t = (dy+1)*3 + (dx+1)
T = {(dy, dx): (dy + 1) * 3 + (dx + 1) for dy in (-1, 0, 1) for dx in (-1, 0, 1)}


def host_prep(kv_w, kv_dw_w, q_w, q_dw_w, proj_w, temperature):
    """Host-side weight transforms (all tiny). Returns dict of extra device inputs."""
    import ml_dtypes
    f8 = lambda a: np.asarray(a, dtype=ml_dtypes.float8_e4m3)
    bf = lambda a: np.asarray(a, dtype=ml_dtypes.bfloat16)

    kv_w = kv_w.astype(np.float64)
    q_w = q_w.astype(np.float64)
    q_dw_w = q_dw_w.astype(np.float64)
    proj_w = proj_w.astype(np.float64)

    # fused dense conv: W_eff[o, j, tap] = sum_i q_dw_w[o,i,tap] * q_w[i,j]
    weff = np.einsum("oiyx,ij->ojyx", q_dw_w, q_w[:, :, 0, 0]).reshape(C, C, 9)
    weff = weff * S_WEFF  # [o, j, tap]

    # ---- q-conv DR moving weights ----
    # hi chunk (contract ch j in 0:128), 5 MM blocks:
    #   MM0..2: row-pair (dy=-1,dy=0) at dx=-1,0,+1 -> slots (T[-1,dx], T[0,dx])
    #   MM3: region pair -> slots (T[1,0], T[1,1])
    #   MM4: plain single -> T[1,-1]
    wqh = np.zeros((128, 2, 5 * C), np.float64)
    for b, dx in enumerate((-1, 0, 1)):
        wqh[:, 0, b * C:(b + 1) * C] = weff[:, 0:128, T[(-1, dx)]].T
        wqh[:, 1, b * C:(b + 1) * C] = weff[:, 0:128, T[(0, dx)]].T
    wqh[:, 0, 3 * C:4 * C] = weff[:, 0:128, T[(1, 0)]].T
    wqh[:, 1, 3 * C:4 * C] = weff[:, 0:128, T[(1, 1)]].T
    wqh[:, 0, 4 * C:5 * C] = weff[:, 0:128, T[(1, -1)]].T

    # lo chunk (contract ch j in 128:192) via partition-dup tile:
    #   partitions p<64 = ch 128+p (cols at +1), p>=64 = ch 128+(p-64) (cols at 0, dx+1)
    #   L0 (block0, quad): p<64 slots (T[-1,-1], T[0,-1]); p>=64 slots (T[-1,0], T[0,0])
    #   L1 (block1): p<64 slots (T[-1,1], T[0,1]); p>=64 ZERO
    #   L2 (block2, plain): p<64 T[1,-1]; p>=64 T[1,0]
    #   L3 (block3, plain, p>=64 only, 64 contract): T[1,1]
    wql = np.zeros((128, 2, 4 * C), np.float64)
    lo = weff[:, 128:192, :]  # [o, 64, 9]
    wql[0:64, 0, 0:C] = lo[:, :, T[(-1, -1)]].T
    wql[0:64, 1, 0:C] = lo[:, :, T[(0, -1)]].T
    wql[64:128, 0, 0:C] = lo[:, :, T[(-1, 0)]].T
    wql[64:128, 1, 0:C] = lo[:, :, T[(0, 0)]].T
    wql[0:64, 0, C:2 * C] = lo[:, :, T[(-1, 1)]].T
    wql[0:64, 1, C:2 * C] = lo[:, :, T[(0, 1)]].T
    wql[0:64, 0, 2 * C:3 * C] = lo[:, :, T[(1, -1)]].T
    wql[64:128, 0, 2 * C:3 * C] = lo[:, :, T[(1, 0)]].T
    wql[0:64, 0, 3 * C:4 * C] = lo[:, :, T[(1, 1)]].T  # used as [64,192] block

    # kv 1x1 stays bf16 (v cannot tolerate fp8): lhsT = W^T [c_in, c_out=2C]
    wkvT = kv_w[:, :, 0, 0].T.copy()  # [192, 384]

    projT = proj_w[:, :, 0, 0].T.copy()  # [c, o]

    dws = kv_dw_w[:, 0].reshape(2 * C, 9).copy()  # [384, 9]
    dwdiag = np.zeros((128, 2 * 9 * 128), np.float64)
    for mc in range(2):          # kv1 chunks 1 and 2 (channels 128..383)
        for t in range(9):
            col = (9 * mc + t) * 128
            dwdiag[np.arange(128), col + np.arange(128)] = dws[128 * (mc + 1):128 * (mc + 2), t]
    tau = np.repeat(np.asarray(temperature, np.float64).reshape(HEADS), CH) * math.log(CH)
    m = np.full((96, 96), -1e9, np.float32)
    m[0:48, 0:48] = 0.0
    m[48:96, 48:96] = 0.0
    return {
        "bmask": m,
        "wqh": f8(wqh.reshape(128, 2 * 5 * C)),
        "wql": f8(wql.reshape(128, 2 * 4 * C)),
        "wkvT": bf(wkvT),
        "projT": bf(projT),
        "dws": dws.astype(np.float32),
        "dwdiag": bf(dwdiag),
        "tau": tau.reshape(C, 1).astype(np.float32),
    }


def build(H=128, debug=False, dw_on_pe=0, reps=1):
    """Build + compile the per-core program. H = image height (rows)."""
    HW = H * W
    NS = H // SR  # stripes
    NCK = HW // 512  # output chunks

    nc = bacc.Bacc("TRN2", target_bir_lowering=False, debug=debug,
                   enable_asserts=False, num_devices=1)
    x = nc.dram_tensor("x", [C, HW], BF16, kind="ExternalInput").ap()
    y = nc.dram_tensor("y", [C, HW], FP8, kind="ExternalInput").ap()
    wqh_d = nc.dram_tensor("wqh", [128, 2 * 5 * C], FP8, kind="ExternalInput").ap()
    wql_d = nc.dram_tensor("wql", [128, 2 * 4 * C], FP8, kind="ExternalInput").ap()
    wkvT = nc.dram_tensor("wkvT", [C, 2 * C], BF16, kind="ExternalInput").ap()
    projT = nc.dram_tensor("projT", [C, C], BF16, kind="ExternalInput").ap()
    dws = nc.dram_tensor("dws", [2 * C, 9], F32, kind="ExternalInput").ap()
    dwdiag = nc.dram_tensor("dwdiag", [128, 2 * 9 * 128], BF16, kind="ExternalInput").ap()
    tau = nc.dram_tensor("tau", [C, 1], F32, kind="ExternalInput").ap()
    out = nc.dram_tensor("out", [C, HW], BF16, kind="ExternalOutput").ap()
    vscr = nc.dram_tensor("vscr", [C, HW], BF16, kind="Internal").ap()
    rscr = nc.dram_tensor("rscr", [1, C], F32, kind="Internal").ap()
    bmask = nc.dram_tensor("bmask", [96, 96], F32, kind="ExternalInput").ap()

    with tile.TileContext(nc) as tc:
        with ExitStack() as ctx:
            wp = ctx.enter_context(tc.tile_pool(name="wp", bufs=1))       # persistent sbuf
            pp = ctx.enter_context(tc.tile_pool(name="pp", bufs=1, space="PSUM"))  # persistent psum

            # ---- weights straight to sbuf ----
            wqh = wp.tile([128, 2, 5 * C], FP8, name="wqh_t")
            nc.sync.dma_start(wqh[:].rearrange("p a b -> p (a b)"), wqh_d)
            wql = wp.tile([128, 2, 4 * C], FP8, name="wql_t")
            nc.sync.dma_start(wql[:].rearrange("p a b -> p (a b)"), wql_d)
            wkv_hi = wp.tile([128, 2 * C], BF16, name="wkv_hi")
            nc.sync.dma_start(wkv_hi[:], wkvT[0:128, :])
            wkv_lo = wp.tile([64, 2 * C], BF16, name="wkv_lo")
            nc.sync.dma_start(wkv_lo[:], wkvT[128:192, :])
            ddg = wp.tile([128, 2 * 9 * 128], BF16, name="ddg")
            nc.sync.dma_start(ddg[:], dwdiag)
            prA = wp.tile([96, C], BF16, name="prA")
            nc.sync.dma_start(prA[:], projT[0:96, :])
            prB = wp.tile([96, C], BF16, name="prB")
            nc.sync.dma_start(prB[:], projT[96:192, :])
            dws_t = wp.tile([128, 9 * 3], F32)  # 3 chunks side by side: [:,9m+t]
            for m in range(3):
                nc.sync.dma_start(dws_t[0:128, 9 * m:9 * m + 9], dws[128 * m:128 * m + 128, :])
            tauA = wp.tile([128, 1], F32)
            nc.sync.dma_start(tauA[:], tau[0:128, :])
            tauB = wp.tile([64, 1], F32)
            nc.sync.dma_start(tauB[:], tau[128:192, :])
            identF = wp.tile([128, 128], F32)
            make_identity(nc, identF[:])

            # preload activation tables used later (Copy/Square/Sqrt/Exp)
            actw = wp.tile([128, 1], F32)
            nc.vector.memset(actw[:], 1.0)
            for fn in (AF.Copy, AF.Square, AF.Sqrt, AF.Exp):
                nc.scalar.activation(actw[:], actw[:], fn)

            # ssq accumulator slots (per stripe), fp32
            ssqA = wp.tile([128, NS], F32)   # k ch 0..127
            ssqB = wp.tile([64, NS], F32)    # k ch 128..191

            # persistent psum: attn raw blocks + q gram
            raw01 = pp.tile([96, 96], F32)
            raw23 = pp.tile([96, 96], F32)
            gq_hi = pp.tile([128, 128], F32)
            gq_lo = pp.tile([64, 64], F32)

            taps = [(dy, dx) for dy in (-1, 0, 1) for dx in (-1, 0, 1)]

            for rep in range(reps):
                with ExitStack() as sctx:
                    sp = sctx.enter_context(tc.tile_pool(name="sp", bufs=2))      # stripe transients
                    qp = sctx.enter_context(tc.tile_pool(name="qp", bufs=34))
                    kvpool = sctx.enter_context(tc.tile_pool(name="kvp", bufs=3))
                    pk = sctx.enter_context(tc.tile_pool(name="pk", bufs=2, space="PSUM"))
                    pq = sctx.enter_context(tc.tile_pool(name="pq", bufs=2, space="PSUM"))

                    for s in range(NS):
                        r_lo = SR * s - 1           # first (halo) image row
                        n_lo = r_lo * W
                        t_a = 1 if s == 0 else 0          # first valid tile row
                        t_b = PR - 1 if s == NS - 1 else PR  # one past last valid tile row

                        # ---- y staging: fp8 padded tiles, direct DMA ----
                        # yhi [128, 36, PWQ]: rows 0:18 region0 (y cols at +1),
                        #                     rows 18:36 region1 (y cols at +0)
                        yhi = sp.tile([128, 2 * PR, PWQ], FP8, tag="yhi", name="yhi")
                        # ylo [128, 18, PWQ]: p<64 ch128+p (cols at +1), p>=64 dup (cols at +0)
                        ylo = sp.tile([128, PR, PWQ], FP8, tag="ylo", name="ylo")
                        if s == 0:
                            nc.gpsimd.memset(yhi[:, 0:1, :], 0.0)
                            nc.gpsimd.memset(yhi[:, PR:PR + 1, :], 0.0)
                            nc.gpsimd.memset(ylo[:, 0:1, :], 0.0)
                        if s == NS - 1:
                            nc.gpsimd.memset(yhi[:, PR - 1:PR, :], 0.0)
                            nc.gpsimd.memset(yhi[:, 2 * PR - 1:2 * PR, :], 0.0)
                            nc.gpsimd.memset(ylo[:, PR - 1:PR, :], 0.0)
                        # pad columns
                        nc.gpsimd.memset(yhi[:, 0:PR, 0:1], 0.0)            # region0 left pad
                        nc.gpsimd.memset(yhi[:, 0:PR, 129:130], 0.0)        # region0 right pad
                        nc.gpsimd.memset(yhi[:, PR:2 * PR, 128:130], 0.0)   # region1 right pad
                        nc.gpsimd.memset(ylo[0:64, :, 0:1], 0.0)
                        nc.gpsimd.memset(ylo[0:64, :, 129:130], 0.0)
                        nc.gpsimd.memset(ylo[64:128, :, 128:130], 0.0)
                        nrows = t_b - t_a
                        src_off = n_lo + t_a * W
                        src_hi = bass.AP(y.tensor, src_off, [[HW, 128], [W, nrows], [1, W]])
                        src_lo = bass.AP(y.tensor, 128 * HW + src_off, [[HW, 64], [W, nrows], [1, W]])
                        nc.sync.dma_start(yhi[:, t_a:t_b, 1:1 + W], src_hi)
                        nc.scalar.dma_start(yhi[:, PR + t_a:PR + t_b, 0:W], src_hi)
                        nc.scalar.dma_start(ylo[0:64, t_a:t_b, 1:1 + W], src_lo)
                        nc.gpsimd.dma_start(ylo[64:128, t_a:t_b, 0:W], src_lo)

                        # ---- x staging: bf16 contiguous tiles ----
                        xbA = sp.tile([128, PR * W], BF16, tag="xbA", name="xbA")
                        xbB = sp.tile([64, PR * W], BF16, tag="xbB", name="xbB")
                        if s == 0:
                            nc.gpsimd.memset(xbA[:, 0:W], 0.0)
                            nc.gpsimd.memset(xbB[:, 0:W], 0.0)
                        if s == NS - 1:
                            nc.gpsimd.memset(xbA[:, (PR - 1) * W:], 0.0)
                            nc.gpsimd.memset(xbB[:, (PR - 1) * W:], 0.0)
                        nc.sync.dma_start(xbA[:, t_a * W:t_b * W],
                                          x[0:128, src_off:src_off + (t_b - t_a) * W])
                        nc.sync.dma_start(xbB[:, t_a * W:t_b * W],
                                          x[128:192, src_off:src_off + (t_b - t_a) * W])

                        # ---- kv 1x1 conv (fp8 DR) -> padded kvp chunks (bf16) ----
                        kvp = []
                        for m in range(3):
                            kvt = kvpool.tile([128, PR, PW], BF16, tag=f"kvp{m}")
                            nc.gpsimd.memset(kvt[:, :, 0:1], 0.0)
                            nc.gpsimd.memset(kvt[:, :, PW - 1:PW], 0.0)
                            kvp.append(kvt)
                            lhs_hi = wkv_hi[:, 128 * m:128 * m + 128]
                            lhs_lo = wkv_lo[:, 128 * m:128 * m + 128]
                            for j in range(0, PR * W, 512):
                                w_ = min(512, PR * W - j)
                                pst = pk.tile([128, 512], F32, tag="pkv", name="pkv")
                                ps = pst[:, 0:w_]
                                nc.tensor.matmul(ps, lhs_hi, xbA[:, j:j + w_], start=True, stop=False)
                                nc.tensor.matmul(ps, lhs_lo, xbB[:, j:j + w_], start=False, stop=True)
                                nc.scalar.copy(
                                    kvt[:, j // W:j // W + w_ // W, 1:1 + W],
                                    ps.rearrange("p (a b) -> p a b", b=W))

                        # ---- depthwise 3x3 (DVE, optionally some chunks on PE) ----
                        kA = sp.tile([128, SR, W], BF16, tag="kA")
                        kvmid = sp.tile([128, SR, W], BF16, tag="kvmid")
                        vB = sp.tile([128, SR, W], BF16, tag="vB")
                        douts = [kA, kvmid, vB]
                        for m in range(3 - dw_on_pe):
                            dst = douts[m]
                            for ti, (dy, dx) in enumerate(taps):
                                sc = dws_t[:, 9 * m + ti:9 * m + ti + 1]
                                src = kvp[m][:, 1 + dy:1 + SR + dy, 1 + dx:1 + dx + W]
                                d = dst[:, :, :]
                                if ti == 0:
                                    nc.vector.tensor_scalar_mul(d, src, sc)
                                else:
                                    tmp = sp.tile([128, SR, W], BF16, tag=f"dwtmp", name="dwtmp")
                                    nc.vector.tensor_scalar_mul(tmp[:], src, sc)
                                    nc.vector.tensor_add(d, d, tmp[:])
                        for m in range(3 - dw_on_pe, 3):
                            dst = douts[m]
                            for jr in range(0, SR, 4):
                                pdw = pk.tile([128, 512], F32, tag="pkv", name="pdw")
                                for ti, (dy, dx) in enumerate(taps):
                                    lhs = ddg[:, (9 * (m - 1) + ti) * 128:(9 * (m - 1) + ti + 1) * 128]
                                    rhs = kvp[m][:, 1 + jr + dy:1 + jr + dy + 4, 1 + dx:1 + dx + W]
                                    nc.tensor.matmul(pdw[:], lhs, rhs, start=(ti == 0), stop=(ti == 8))
                                nc.vector.tensor_copy(
                                    dst[:, jr:jr + 4, :],
                                    pdw[:].rearrange("p (a b) -> p a b", b=W))

                        # ---- ssq_k ----
                        scr = sp.tile([128, SR, W], BF16, tag="dwtmp", name="scr")
                        nc.scalar.activation(scr[:], kA[:], AF.Square, accum_out=ssqA[:, s:s + 1])
                        nc.scalar.activation(scr[0:64], kvmid[0:64], AF.Square, accum_out=ssqB[:, s:s + 1])

                        # ---- v spill ----
                        nsl = slice(SR * s * W, SR * s * W + SR * W)
                        nc.sync.dma_start(vscr[0:64, nsl], kvmid[64:128].rearrange("p a b -> p (a b)"))
                        nc.sync.dma_start(vscr[64:192, nsl], vB[:].rearrange("p a b -> p (a b)"))

                        # ---- k transpose: kt[p, r, c] = k[c, r*W + p] ----
                        kt = sp.tile([128, SR, C], BF16, tag="kt")
                        nc.sync.dma_start_transpose(kt[:, :, 0:128], kA[:].rearrange("p a b -> p (a b)"))
                        nc.sync.dma_start_transpose(kt[:, :, 128:192], kvmid[0:64].rearrange("p a b -> p (a b)"))

                        # ---- q conv (fp8 DR) + attn/gram accumulation ----
                        def attn_mms(qsb, r, kt_, f, l):
                            nc.tensor.matmul(gq_hi[:], qsb[:, 0:128], qsb[:, 0:128], start=f, stop=l)
                            nc.tensor.matmul(gq_lo[:], qsb[:, 128:192], qsb[:, 128:192], start=f, stop=l)
                            nc.tensor.matmul(raw01[:], qsb[:, 0:96], kt_[:, r, 0:96], start=f, stop=l)
                            nc.tensor.matmul(raw23[:], qsb[:, 96:192], kt_[:, r, 96:192], start=f, stop=l)

                        cur_q = []
                        for r in range(SR):
                            psq = pq.tile([128, C], F32, tag="psq")
                            # hi: 3 row-pair DR MMs (dx=-1,0,+1)
                            for b in range(3):
                                nc.tensor.matmul(psq, yhi[:, r:r + 2, b:b + 128],
                                                 wqh[:, :, b * C:(b + 1) * C],
                                                 start=(b == 0), stop=False, perf_mode=DR)
                            # hi: region-pair (dy=+1: dx=0 & dx=+1)
                            nc.tensor.matmul(psq, yhi[:, (r + 2):(r + 21):18, 1:129],
                                             wqh[:, :, 3 * C:4 * C],
                                             start=False, stop=False, perf_mode=DR)
                            # hi: plain single (dy=+1, dx=-1)
                            nc.tensor.matmul(psq, yhi[:, r + 2, 0:128],
                                             wqh[:, 0, 4 * C:5 * C],
                                             start=False, stop=False)
                            # lo: quad DR (block0), half DR (block1)
                            nc.tensor.matmul(psq, ylo[:, r:r + 2, 0:128],
                                             wql[:, :, 0:C],
                                             start=False, stop=False, perf_mode=DR)
                            nc.tensor.matmul(psq, ylo[:, r:r + 2, 2:130],
                                             wql[:, :, C:2 * C],
                                             start=False, stop=False, perf_mode=DR)
                            # lo: plain (dy=+1 row, 2 taps via dup)
                            nc.tensor.matmul(psq, ylo[:, r + 2, 0:128],
                                             wql[:, 0, 2 * C:3 * C],
                                             start=False, stop=False)
                            # lo: plain 64p (dy=+1, dx=+1)
                            nc.tensor.matmul(psq, ylo[64:128, r + 2, 1:129],
                                             wql[64:128, 0, 3 * C:4 * C],
                                             start=False, stop=True)
                            qsb = qp.tile([128, C], BF16, tag="qsb")
                            nc.scalar.copy(qsb[:], psq[:])
                            cur_q.append(qsb)
                            if s > 0:
                                attn_mms(prev_q[r], r, prev_kt,
                                         f=(s == 1 and r == 0), l=False)
                        prev_q, prev_kt = cur_q, kt

                    # drain attn for the last stripe
                    for r in range(SR):
                        attn_mms(prev_q[r], r, prev_kt, f=False, l=(r == SR - 1))

                # ================= phase 2: softmax + MT =================
                with ExitStack() as sctx2:
                    s2 = sctx2.enter_context(tc.tile_pool(name="s2", bufs=1))
                    p2 = sctx2.enter_context(tc.tile_pool(name="p2", bufs=1, space="PSUM"))

                    # ssq_q from gram diagonals
                    scr2 = s2.tile([128, 128], F32)
                    ssqqA = s2.tile([128, 1], F32)
                    nc.vector.scalar_tensor_tensor(scr2[:], gq_hi[:], 1.0, identF[:],
                                                   AL.mult, AL.mult, accum_out=ssqqA[:])
                    scr2b = s2.tile([64, 64], F32)
                    ssqqB = s2.tile([64, 1], F32)
                    nc.vector.scalar_tensor_tensor(scr2b[:], gq_lo[:], 1.0, identF[0:64, 0:64],
                                                   AL.mult, AL.mult, accum_out=ssqqB[:])
                    # ssq_k totals
                    sskA = s2.tile([128, 1], F32)
                    nc.vector.reduce_sum(sskA[:], ssqA[:], axis=mybir.AxisListType.X)
                    sskB = s2.tile([64, 1], F32)
                    nc.vector.reduce_sum(sskB[:], ssqB[:], axis=mybir.AxisListType.X)

                    def rsqrt(dst, src):
                        nc.vector.reciprocal(dst, src)
                        nc.scalar.activation(dst, dst, AF.Sqrt)

                    rqA = s2.tile([128, 1], F32, name="rqA")
                    rsqrt(rqA[:], ssqqA[:])
                    rqB = s2.tile([64, 1], F32, name="rqB")
                    rsqrt(rqB[:], ssqqB[:])
                    rkA = s2.tile([128, 1], F32, name="rkA")
                    rsqrt(rkA[:], sskA[:])
                    rkB = s2.tile([64, 1], F32, name="rkB")
                    rsqrt(rkB[:], sskB[:])
                    # rq * tau
                    nc.vector.tensor_mul(rqA[:], rqA[:], tauA[:])
                    nc.vector.tensor_mul(rqB[:], rqB[:], tauB[:])

                    # rk rows [1, 96] then broadcast [96, 96] via K=1 matmul with ones
                    nc.sync.dma_start(rscr[0:1, 0:128].rearrange("a b -> b a"), rkA[:])
                    nc.sync.dma_start(rscr[0:1, 128:192].rearrange("a b -> b a"), rkB[:])
                    rkrow = s2.tile([1, 192], F32)
                    nc.sync.dma_start(rkrow[:], rscr)
                    rkrow_b = s2.tile([1, 192], BF16)
                    nc.vector.tensor_copy(rkrow_b[:], rkrow[:])
                    ones1 = s2.tile([1, 96], BF16)
                    nc.vector.memset(ones1[:], 1.0)
                    rkb01p = p2.tile([96, 96], F32)
                    nc.tensor.matmul(rkb01p[:], ones1[:], rkrow_b[0:1, 0:96], start=True, stop=True)
                    rkb23p = p2.tile([96, 96], F32)
                    nc.tensor.matmul(rkb23p[:], ones1[:], rkrow_b[0:1, 96:192], start=True, stop=True)

                    # logits = raw * (rq*tau) * rk
                    l01 = s2.tile([96, 96], F32)
                    nc.scalar.activation(l01[:], raw01[:], AF.Copy, scale=rqA[0:96, :])
                    nc.vector.tensor_mul(l01[:], l01[:], rkb01p[:])
                    l23 = s2.tile([96, 96], F32)
                    rq23 = s2.tile([96, 1], F32)
                    nc.sync.dma_start(rq23[0:32, :], rqA[96:128, :])
                    nc.sync.dma_start(rq23[32:96, :], rqB[:])
                    nc.scalar.activation(l23[:], raw23[:], AF.Copy, scale=rq23[:])
                    nc.vector.tensor_mul(l23[:], l23[:], rkb23p[:])

                    # softmax per head-pair with additive block mask -> blockdiag bd (bf16)
                    msk = s2.tile([96, 96], F32)
                    nc.sync.dma_start(msk[:], bmask)
                    bd01 = s2.tile([96, 96], BF16)
                    bd23 = s2.tile([96, 96], BF16)
                    for hb, (lt, bd) in enumerate(((l01, bd01), (l23, bd23))):
                        nc.vector.tensor_add(lt[:], lt[:], msk[:])
                        mx = s2.tile([96, 1], F32, tag=f"mx{hb}", name=f"mx{hb}")
                        nc.vector.reduce_max(mx[:], lt[:], axis=mybir.AxisListType.X)
                        nc.vector.tensor_scalar_mul(mx[:], mx[:], -1.0)
                        ex = s2.tile([96, 96], F32, tag=f"ex{hb}", name=f"ex{hb}")
                        rs = s2.tile([96, 1], F32, tag=f"rs{hb}", name=f"rs{hb}")
                        nc.scalar.activation(ex[:], lt[:], AF.Exp, bias=mx[:], accum_out=rs[:])
                        nc.vector.reciprocal(rs[:], rs[:])
                        nc.vector.tensor_scalar_mul(bd[:], ex[:], rs[:])

                    # MT[d, o] = sum_c attn[c, d] * projT[c, o]
                    mt_hi_p = p2.tile([96, C], F32)
                    nc.tensor.matmul(mt_hi_p[:], bd01[:], prA[:], start=True, stop=True)
                    mt_lo_p = p2.tile([96, C], F32)
                    nc.tensor.matmul(mt_lo_p[:], bd23[:], prB[:], start=True, stop=True)
                    mt_hi = wp.tile([96, C], BF16)
                    nc.scalar.copy(mt_hi[:], mt_hi_p[:])
                    mt_lo = wp.tile([96, C], BF16)
                    nc.scalar.copy(mt_lo[:], mt_lo_p[:])

                # ================= phase 3: out = MT.T @ v, streamed =================
                with ExitStack() as sctx3:
                    s3 = sctx3.enter_context(tc.tile_pool(name="s3", bufs=3))
                    p3 = sctx3.enter_context(tc.tile_pool(name="p3", bufs=2, space="PSUM"))
                    for g in range(NCK // 4):
                        gsl = slice(2048 * g, 2048 * g + 2048)
                        vhi = s3.tile([96, 2048], BF16, tag="vhi")
                        nc.sync.dma_start(vhi[:], vscr[0:96, gsl])
                        vlo = s3.tile([96, 2048], BF16, tag="vlo")
                        nc.sync.dma_start(vlo[:], vscr[96:192, gsl])
                        o1g = s3.tile([128, 2048], BF16, tag="o1")
                        o2g = s3.tile([64, 2048], BF16, tag="o2")
                        for jj in range(4):
                            vsl = slice(512 * jj, 512 * jj + 512)
                            f1 = p3.tile([128, 512], F32, tag="f1")
                            nc.tensor.matmul(f1[:], mt_hi[:, 0:128], vhi[:, vsl], start=True, stop=False)
                            nc.tensor.matmul(f1[:], mt_lo[:, 0:128], vlo[:, vsl], start=False, stop=True)
                            f2 = p3.tile([64, 512], F32, tag="f2")
                            nc.tensor.matmul(f2[:], mt_hi[:, 128:192], vhi[:, vsl], start=True, stop=False)
                            nc.tensor.matmul(f2[:], mt_lo[:, 128:192], vlo[:, vsl], start=False, stop=True)
                            nc.vector.tensor_copy(o1g[:, vsl], f1[:])
                            nc.scalar.copy(o2g[:, vsl], f2[:])
                        nc.scalar.dma_start(out[0:128, gsl], o1g[:])
                        nc.sync.dma_start(out[128:192, gsl], o2g[:])

    nc.compile()
    return nc


# ======================= harness entry point =======================
B = 8
H = 128
_NC = None


def _get_nc():
    global _NC
    if _NC is None:
        _NC = build(H=H)
    return _NC


def _make_in_maps(inputs, H=H):
    import ml_dtypes
    x = np.asarray(np.ascontiguousarray(inputs["x"], np.float32), dtype=ml_dtypes.bfloat16)
    y = np.asarray(np.ascontiguousarray(inputs["y"], np.float32), dtype=ml_dtypes.float8_e4m3)
    prep = host_prep(inputs["kv_w"], inputs["kv_dw_w"], inputs["q_w"],
                     inputs["q_dw_w"], inputs["proj_w"], inputs["temperature"])
    maps = []
    for b in range(x.shape[0]):
        m = {"x": x[b].reshape(C, H * W), "y": y[b].reshape(C, H * W)}
        m.update(prep)
        maps.append(m)
    return maps


def _run(inputs, trace=False, trace_kwargs=None):
    from concourse.bass_utils import run_bass_kernel_spmd
    nc = _get_nc()
    res = run_bass_kernel_spmd(nc, _make_in_maps(inputs), core_ids=list(range(B)),
                               trace=trace, trace_kwargs=trace_kwargs or {})
    out = np.stack([np.asarray(res.results[b]["out"]).astype(np.float32).reshape(C, H, W)
                    for b in range(B)])
    return out, res


def kernel(**inputs) -> np.ndarray:
    out, _ = _run(inputs, trace=False)
    return out


# revision 4
# speedup vs baseline: 1.0428x; 1.0428x over previous
import math
from contextlib import ExitStack

import numpy as np

import concourse.bass as bass
import concourse.tile as tile
from concourse import bacc, mybir
from concourse.masks import make_identity

F32 = mybir.dt.float32
BF16 = mybir.dt.bfloat16
FP8 = mybir.dt.float8e4
AL = mybir.AluOpType
AF = mybir.ActivationFunctionType
DR = mybir.MatmulPerfMode.DoubleRow

C = 192          # channels
HEADS = 4
CH = C // HEADS  # 48
W = 128          # image width
SR = 16          # rows per stripe
PW = W + 2       # padded width (bf16 kvp tiles)
PWQ = 144        # padded width for fp8 q staging (16B-aligned pair strides)
PR = SR + 2      # padded rows per stripe

S_WEFF = 256.0   # fp8 scale for q-conv weights (cancels in l2-norm)
S_KV = 64.0      # (unused: kv conv stays bf16 -- v cannot tolerate fp8)

# tap index t = (dy+1)*3 + (dx+1)
T = {(dy, dx): (dy + 1) * 3 + (dx + 1) for dy in (-1, 0, 1) for dx in (-1, 0, 1)}


def host_prep(kv_w, kv_dw_w, q_w, q_dw_w, proj_w, temperature):
    """Host-side weight transforms (all tiny). Returns dict of extra device inputs."""
    import ml_dtypes
    f8 = lambda a: np.asarray(a, dtype=ml_dtypes.float8_e4m3)
    bf = lambda a: np.asarray(a, dtype=ml_dtypes.bfloat16)

    kv_w = kv_w.astype(np.float64)
    q_w = q_w.astype(np.float64)
    q_dw_w = q_dw_w.astype(np.float64)
    proj_w = proj_w.astype(np.float64)

    # fused dense conv: W_eff[o, j, tap] = sum_i q_dw_w[o,i,tap] * q_w[i,j]
    weff = np.einsum("oiyx,ij->ojyx", q_dw_w, q_w[:, :, 0, 0]).reshape(C, C, 9)
    weff = weff * S_WEFF  # [o, j, tap]

    # ---- q-conv DR moving weights ----
    # hi chunk (contract ch j in 0:128), 5 MM blocks:
    #   MM0..2: row-pair (dy=-1,dy=0) at dx=-1,0,+1 -> slots (T[-1,dx], T[0,dx])
    #   MM3: region pair -> slots (T[1,0], T[1,1])
    #   MM4: plain single -> T[1,-1]
    wqh = np.zeros((128, 2, 5 * C), np.float64)
    for b, dx in enumerate((-1, 0, 1)):
        wqh[:, 0, b * C:(b + 1) * C] = weff[:, 0:128, T[(-1, dx)]].T
        wqh[:, 1, b * C:(b + 1) * C] = weff[:, 0:128, T[(0, dx)]].T
    wqh[:, 0, 3 * C:4 * C] = weff[:, 0:128, T[(1, 0)]].T
    wqh[:, 1, 3 * C:4 * C] = weff[:, 0:128, T[(1, 1)]].T
    wqh[:, 0, 4 * C:5 * C] = weff[:, 0:128, T[(1, -1)]].T

    # lo chunk (contract ch j in 128:192) via partition-dup tile:
    #   partitions p<64 = ch 128+p (cols at +1), p>=64 = ch 128+(p-64) (cols at 0, dx+1)
    #   L0 (block0, quad): p<64 slots (T[-1,-1], T[0,-1]); p>=64 slots (T[-1,0], T[0,0])
    #   L1 (block1): p<64 slots (T[-1,1], T[0,1]); p>=64 ZERO
    #   L2 (block2, plain): p<64 T[1,-1]; p>=64 T[1,0]
    #   L3 (block3, plain, p>=64 only, 64 contract): T[1,1]
    wql = np.zeros((128, 2, 4 * C), np.float64)
    lo = weff[:, 128:192, :]  # [o, 64, 9]
    wql[0:64, 0, 0:C] = lo[:, :, T[(-1, -1)]].T
    wql[0:64, 1, 0:C] = lo[:, :, T[(0, -1)]].T
    wql[64:128, 0, 0:C] = lo[:, :, T[(-1, 0)]].T
    wql[64:128, 1, 0:C] = lo[:, :, T[(0, 0)]].T
    wql[0:64, 0, C:2 * C] = lo[:, :, T[(-1, 1)]].T
    wql[0:64, 1, C:2 * C] = lo[:, :, T[(0, 1)]].T
    wql[0:64, 0, 2 * C:3 * C] = lo[:, :, T[(1, -1)]].T
    wql[64:128, 0, 2 * C:3 * C] = lo[:, :, T[(1, 0)]].T
    wql[0:64, 0, 3 * C:4 * C] = lo[:, :, T[(1, 1)]].T  # used as [64,192] block

    # kv 1x1: chunk m=0 (pure k) in fp8-DR; chunks 1,2 bf16 (v cannot tolerate fp8)
    wkvT = kv_w[:, :, 0, 0].T.copy()  # [192, 384]
    wkv8 = np.zeros((96, 2, 128), np.float64)  # contract slot (p,i) = in-ch p+96i
    wkv8[:, 0, :] = (kv_w[0:128, 0:96, 0, 0] * S_KV).T
    wkv8[:, 1, :] = (kv_w[0:128, 96:192, 0, 0] * S_KV).T

    projT = proj_w[:, :, 0, 0].T.copy()  # [c, o]

    dws = kv_dw_w[:, 0].reshape(2 * C, 9).copy()  # [384, 9]
    dwdiag = np.zeros((128, 2 * 9 * 128), np.float64)
    for mc in range(2):          # kv1 chunks 1 and 2 (channels 128..383)
        for t in range(9):
            col = (9 * mc + t) * 128
            dwdiag[np.arange(128), col + np.arange(128)] = dws[128 * (mc + 1):128 * (mc + 2), t]
    tau = np.repeat(np.asarray(temperature, np.float64).reshape(HEADS), CH) * math.log(CH)
    m = np.full((96, 96), -1e9, np.float32)
    m[0:48, 0:48] = 0.0
    m[48:96, 48:96] = 0.0
    return {
        "bmask": m,
        "wqh": f8(wqh.reshape(128, 2 * 5 * C)),
        "wql": f8(wql.reshape(128, 2 * 4 * C)),
        "wkvT": bf(wkvT),
        "wkv8": f8(wkv8.reshape(96, 2 * 128)),
        "projT": bf(projT),
        "dws": dws.astype(np.float32),
        "dwdiag": bf(dwdiag),
        "tau": tau.reshape(C, 1).astype(np.float32),
    }


def build(H=128, debug=False, dw_on_pe=0, reps=1, ablate=(), bufs=None):
    """Build + compile the per-core program. H = image height (rows)."""
    HW = H * W
    NS = H // SR  # stripes
    NCK = HW // 512  # output chunks

    nc = bacc.Bacc("TRN2", target_bir_lowering=False, debug=debug,
                   enable_asserts=False, num_devices=1)
    x = nc.dram_tensor("x", [C, HW], BF16, kind="ExternalInput").ap()
    x8 = nc.dram_tensor("x8", [C, HW], FP8, kind="ExternalInput").ap()
    wkv8_d = nc.dram_tensor("wkv8", [96, 2 * 128], FP8, kind="ExternalInput").ap()
    y = nc.dram_tensor("y", [C, HW], FP8, kind="ExternalInput").ap()
    wqh_d = nc.dram_tensor("wqh", [128, 2 * 5 * C], FP8, kind="ExternalInput").ap()
    wql_d = nc.dram_tensor("wql", [128, 2 * 4 * C], FP8, kind="ExternalInput").ap()
    wkvT = nc.dram_tensor("wkvT", [C, 2 * C], BF16, kind="ExternalInput").ap()
    projT = nc.dram_tensor("projT", [C, C], BF16, kind="ExternalInput").ap()
    dws = nc.dram_tensor("dws", [2 * C, 9], F32, kind="ExternalInput").ap()
    dwdiag = nc.dram_tensor("dwdiag", [128, 2 * 9 * 128], BF16, kind="ExternalInput").ap()
    tau = nc.dram_tensor("tau", [C, 1], F32, kind="ExternalInput").ap()
    out = nc.dram_tensor("out", [C, HW], BF16, kind="ExternalOutput").ap()
    vscr = nc.dram_tensor("vscr", [C, HW], BF16, kind="Internal").ap()
    rscr = nc.dram_tensor("rscr", [1, C], F32, kind="Internal").ap()
    bmask = nc.dram_tensor("bmask", [96, 96], F32, kind="ExternalInput").ap()

    with tile.TileContext(nc) as tc:
        with ExitStack() as ctx:
            wp = ctx.enter_context(tc.tile_pool(name="wp", bufs=1))       # persistent sbuf
            pp = ctx.enter_context(tc.tile_pool(name="pp", bufs=1, space="PSUM"))  # persistent psum

            # ---- weights straight to sbuf ----
            wqh = wp.tile([128, 2, 5 * C], FP8, name="wqh_t")
            nc.sync.dma_start(wqh[:].rearrange("p a b -> p (a b)"), wqh_d)
            wql = wp.tile([128, 2, 4 * C], FP8, name="wql_t")
            nc.sync.dma_start(wql[:].rearrange("p a b -> p (a b)"), wql_d)
            wkv8_t = wp.tile([96, 2, 128], FP8, name="wkv8_t")
            nc.sync.dma_start(wkv8_t[:].rearrange("p a b -> p (a b)"), wkv8_d)
            wkv_hi = wp.tile([128, 2 * C], BF16, name="wkv_hi")
            nc.sync.dma_start(wkv_hi[:], wkvT[0:128, :])
            wkv_lo = wp.tile([64, 2 * C], BF16, name="wkv_lo")
            nc.sync.dma_start(wkv_lo[:], wkvT[128:192, :])
            ddg = wp.tile([128, 2 * 9 * 128], BF16, name="ddg")
            nc.sync.dma_start(ddg[:], dwdiag)
            prA = wp.tile([96, C], BF16, name="prA")
            nc.sync.dma_start(prA[:], projT[0:96, :])
            prB = wp.tile([96, C], BF16, name="prB")
            nc.sync.dma_start(prB[:], projT[96:192, :])
            dws_t = wp.tile([128, 9 * 3], F32)  # 3 chunks side by side: [:,9m+t]
            for m in range(3):
                nc.sync.dma_start(dws_t[0:128, 9 * m:9 * m + 9], dws[128 * m:128 * m + 128, :])
            tauA = wp.tile([128, 1], F32)
            nc.sync.dma_start(tauA[:], tau[0:128, :])
            tauB = wp.tile([64, 1], F32)
            nc.sync.dma_start(tauB[:], tau[128:192, :])
            identF = wp.tile([128, 128], F32)
            make_identity(nc, identF[:])

            # preload activation tables used later (Copy/Square/Sqrt/Exp)
            actw = wp.tile([128, 1], F32)
            nc.vector.memset(actw[:], 1.0)
            for fn in (AF.Copy, AF.Square, AF.Sqrt, AF.Exp):
                nc.scalar.activation(actw[:], actw[:], fn)

            # ssq accumulator slots (per stripe), fp32
            ssqA = wp.tile([128, NS], F32)   # k ch 0..127
            ssqB = wp.tile([64, NS], F32)    # k ch 128..191

            # persistent psum: attn raw blocks + q gram packed into ONE bank
            accb = pp.tile([128, 512], F32, name="accb")
            raw01 = accb[0:96, 0:96]
            raw23 = accb[0:96, 96:192]
            gq_hi = accb[:, 192:320]
            gq_lo = accb[0:64, 320:384]

            taps = [(dy, dx) for dy in (-1, 0, 1) for dx in (-1, 0, 1)]

            for rep in range(reps):
                with ExitStack() as sctx:
                    bu = bufs if bufs is not None else {"pk": 3, "pq": 3}
                    sp = sctx.enter_context(tc.tile_pool(name="sp", bufs=bu.get("sp", 2)))      # stripe transients
                    qp = sctx.enter_context(tc.tile_pool(name="qp", bufs=bu.get("qp", 34)))
                    kvpool = sctx.enter_context(tc.tile_pool(name="kvp", bufs=bu.get("kvp", 3)))
                    pk = sctx.enter_context(tc.tile_pool(name="pk", bufs=bu.get("pk", 2), space="PSUM"))
                    pq = sctx.enter_context(tc.tile_pool(name="pq", bufs=bu.get("pq", 2), space="PSUM"))

                    for s in range(NS):
                        r_lo = SR * s - 1           # first (halo) image row
                        n_lo = r_lo * W
                        t_a = 1 if s == 0 else 0          # first valid tile row
                        t_b = PR - 1 if s == NS - 1 else PR  # one past last valid tile row

                        # ---- y staging: fp8 padded tiles, direct DMA ----
                        # yhi [128, 36, PWQ]: rows 0:18 region0 (y cols at +1),
                        #                     rows 18:36 region1 (y cols at +0)
                        yhi = sp.tile([128, 2 * PR, PWQ], FP8, tag="yhi", name="yhi")
                        # ylo [128, 18, PWQ]: p<64 ch128+p (cols at +1), p>=64 dup (cols at +0)
                        ylo = sp.tile([128, PR, PWQ], FP8, tag="ylo", name="ylo")
                        if s == 0:
                            nc.gpsimd.memset(yhi[:, 0:1, :], 0.0)
                            nc.gpsimd.memset(yhi[:, PR:PR + 1, :], 0.0)
                            nc.gpsimd.memset(ylo[:, 0:1, :], 0.0)
                        if s == NS - 1:
                            nc.gpsimd.memset(yhi[:, PR - 1:PR, :], 0.0)
                            nc.gpsimd.memset(yhi[:, 2 * PR - 1:2 * PR, :], 0.0)
                            nc.gpsimd.memset(ylo[:, PR - 1:PR, :], 0.0)
                        # pad columns
                        nc.gpsimd.memset(yhi[:, 0:PR, 0:1], 0.0)            # region0 left pad
                        nc.gpsimd.memset(yhi[:, 0:PR, 129:130], 0.0)        # region0 right pad
                        nc.gpsimd.memset(yhi[:, PR:2 * PR, 128:130], 0.0)   # region1 right pad
                        nc.gpsimd.memset(ylo[0:64, :, 0:1], 0.0)
                        nc.gpsimd.memset(ylo[0:64, :, 129:130], 0.0)
                        nc.gpsimd.memset(ylo[64:128, :, 128:130], 0.0)
                        nrows = t_b - t_a
                        src_off = n_lo + t_a * W
                        src_hi = bass.AP(y.tensor, src_off, [[HW, 128], [W, nrows], [1, W]])
                        src_lo = bass.AP(y.tensor, 128 * HW + src_off, [[HW, 64], [W, nrows], [1, W]])
                        if "ystage" not in ablate or s < 3:
                            nc.sync.dma_start(yhi[:, t_a:t_b, 1:1 + W], src_hi)
                            nc.scalar.dma_start(yhi[:, PR + t_a:PR + t_b, 0:W], src_hi)
                            nc.scalar.dma_start(ylo[0:64, t_a:t_b, 1:1 + W], src_lo)
                            nc.gpsimd.dma_start(ylo[64:128, t_a:t_b, 0:W], src_lo)

                        # ---- x staging: bf16 contiguous tiles ----
                        xbA = sp.tile([128, PR * W], BF16, tag="xbA", name="xbA")
                        xbB = sp.tile([64, PR * W], BF16, tag="xbB", name="xbB")
                        if s == 0:
                            nc.gpsimd.memset(xbA[:, 0:W], 0.0)
                            nc.gpsimd.memset(xbB[:, 0:W], 0.0)
                        if s == NS - 1:
                            nc.gpsimd.memset(xbA[:, (PR - 1) * W:], 0.0)
                            nc.gpsimd.memset(xbB[:, (PR - 1) * W:], 0.0)
                        nc.sync.dma_start(xbA[:, t_a * W:t_b * W],
                                          x[0:128, src_off:src_off + (t_b - t_a) * W])
                        nc.sync.dma_start(xbB[:, t_a * W:t_b * W],
                                          x[128:192, src_off:src_off + (t_b - t_a) * W])
                        x84 = sp.tile([96, 2, PR * W], FP8, tag="x84", name="x84")
                        if s == 0:
                            nc.gpsimd.memset(x84[:, :, 0:W], 0.0)
                        if s == NS - 1:
                            nc.gpsimd.memset(x84[:, :, (PR - 1) * W:], 0.0)
                        src_x0 = bass.AP(x8.tensor, src_off, [[HW, 96], [1, nrows * W]])
                        src_x1 = bass.AP(x8.tensor, 96 * HW + src_off, [[HW, 96], [1, nrows * W]])
                        nc.scalar.dma_start(x84[:, 0, t_a * W:t_b * W], src_x0)
                        nc.scalar.dma_start(x84[:, 1, t_a * W:t_b * W], src_x1)

                        # ---- kv 1x1 conv (fp8 DR) -> padded kvp chunks (bf16) ----
                        kvp = []
                        for m in range(3):
                            kvt = kvpool.tile([128, PR, PW], BF16, tag=f"kvp{m}")
                            nc.gpsimd.memset(kvt[:, :, 0:1], 0.0)
                            nc.gpsimd.memset(kvt[:, :, PW - 1:PW], 0.0)
                            kvp.append(kvt)
                            lhs_hi = wkv_hi[:, 128 * m:128 * m + 128]
                            lhs_lo = wkv_lo[:, 128 * m:128 * m + 128]
                            for j in range(0, PR * W, 512):
                                w_ = min(512, PR * W - j)
                                pst = pk.tile([128, 512], F32, tag="pkv", name="pkv")
                                ps = pst[:, 0:w_]
                                if m == 0:
                                    nc.tensor.matmul(ps, wkv8_t[:], x84[:, :, j:j + w_],
                                                     start=True, stop=True, perf_mode=DR)
                                else:
                                    nc.tensor.matmul(ps, lhs_hi, xbA[:, j:j + w_], start=True, stop=False)
                                    nc.tensor.matmul(ps, lhs_lo, xbB[:, j:j + w_], start=False, stop=True)
                                if "kvcopy" not in ablate or s < 3:
                                    nc.scalar.copy(
                                        kvt[:, j // W:j // W + w_ // W, 1:1 + W],
                                        ps.rearrange("p (a b) -> p a b", b=W))

                        # ---- depthwise 3x3 (DVE, optionally some chunks on PE) ----
                        kA = sp.tile([128, SR, W], BF16, tag="kA")
                        kvmid = sp.tile([128, SR, W], BF16, tag="kvmid")
                        vB = sp.tile([128, SR, W], BF16, tag="vB")
                        douts = [kA, kvmid, vB]
                        if "dw" in ablate:
                            for dsta in douts:
                                nc.vector.memset(dsta[:], 0.01)
                        for m in range(0 if "dw" in ablate else (3 - dw_on_pe)):
                            dst = douts[m]
                            for ti, (dy, dx) in enumerate(taps):
                                sc = dws_t[:, 9 * m + ti:9 * m + ti + 1]
                                src = kvp[m][:, 1 + dy:1 + SR + dy, 1 + dx:1 + dx + W]
                                d = dst[:, :, :]
                                if ti == 0:
                                    nc.vector.tensor_scalar_mul(d, src, sc)
                                else:
                                    tmp = sp.tile([128, SR, W], BF16, tag=f"dwtmp", name="dwtmp")
                                    nc.vector.tensor_scalar_mul(tmp[:], src, sc)
                                    nc.vector.tensor_add(d, d, tmp[:])
                        for m in range(3 - dw_on_pe, 3):
                            dst = douts[m]
                            for jr in range(0, SR, 4):
                                pdw = pk.tile([128, 512], F32, tag="pkv", name="pdw")
                                for ti, (dy, dx) in enumerate(taps):
                                    lhs = ddg[:, (9 * (m - 1) + ti) * 128:(9 * (m - 1) + ti + 1) * 128]
                                    rhs = kvp[m][:, 1 + jr + dy:1 + jr + dy + 4, 1 + dx:1 + dx + W]
                                    nc.tensor.matmul(pdw[:], lhs, rhs, start=(ti == 0), stop=(ti == 8))
                                nc.vector.tensor_copy(
                                    dst[:, jr:jr + 4, :],
                                    pdw[:].rearrange("p (a b) -> p a b", b=W))

                        # ---- ssq_k ----
                        scr = sp.tile([128, SR, W], BF16, tag="dwtmp", name="scr")
                        if "ssq" not in ablate or True:
                            nc.scalar.activation(scr[:], kA[:], AF.Square, accum_out=ssqA[:, s:s + 1])
                            nc.scalar.activation(scr[0:64], kvmid[0:64], AF.Square, accum_out=ssqB[:, s:s + 1])
                        elif s == 0:
                            nc.vector.memset(ssqA[:], 1.0)
                            nc.vector.memset(ssqB[:], 1.0)

                        # ---- v spill ----
                        nsl = slice(SR * s * W, SR * s * W + SR * W)
                        nc.sync.dma_start(vscr[0:64, nsl], kvmid[64:128].rearrange("p a b -> p (a b)"))
                        nc.sync.dma_start(vscr[64:192, nsl], vB[:].rearrange("p a b -> p (a b)"))

                        # ---- k transpose: kt[p, r, c] = k[c, r*W + p] ----
                        kt = sp.tile([128, SR, C], BF16, tag="kt")
                        nc.sync.dma_start_transpose(kt[:, :, 0:128], kA[:].rearrange("p a b -> p (a b)"))
                        nc.sync.dma_start_transpose(kt[:, :, 128:192], kvmid[0:64].rearrange("p a b -> p (a b)"))

                        # ---- q conv (fp8 DR) + attn/gram accumulation ----
                        def attn_mms(qsb, r, kt_, f, l):
                            nc.tensor.matmul(gq_hi, qsb[:, 0:128], qsb[:, 0:128], start=f, stop=l)
                            nc.tensor.matmul(gq_lo, qsb[:, 128:192], qsb[:, 128:192], start=f, stop=l)
                            nc.tensor.matmul(raw01, qsb[:, 0:96], kt_[:, r, 0:96], start=f, stop=l)
                            nc.tensor.matmul(raw23, qsb[:, 96:192], kt_[:, r, 96:192], start=f, stop=l)

                        cur_q = []
                        for r0 in range(0, SR, 2):
                          psq2 = pq.tile([128, 2 * C], F32, tag="psq", name="psq2")
                          qsb2 = qp.tile([128, 2 * C], BF16, tag="qsb", name="qsb2")
                          for r in (r0, r0 + 1):
                            psq = psq2[:, (r - r0) * C:(r - r0) * C + C]
                            if "qconv" in ablate:
                                nc.tensor.matmul(psq, yhi[:, r:r + 2, 0:128],
                                                 wqh[:, :, 0:C],
                                                 start=True, stop=True, perf_mode=DR)
                                qsb = qsb2[:, (r - r0) * C:(r - r0) * C + C]
                                cur_q.append(qsb)
                                if r == r0 + 1:
                                    nc.scalar.copy(qsb2[:], psq2[:])
                                if s > 0:
                                    attn_mms(prev_q[r], r, prev_kt,
                                             f=(s == 1 and r == 0), l=False)
                                continue
                            # hi: 3 row-pair DR MMs (dx=-1,0,+1)
                            for b in range(3):
                                nc.tensor.matmul(psq, yhi[:, r:r + 2, b:b + 128],
                                                 wqh[:, :, b * C:(b + 1) * C],
                                                 start=(b == 0), stop=False, perf_mode=DR)
                            # hi: region-pair (dy=+1: dx=0 & dx=+1)
                            nc.tensor.matmul(psq, yhi[:, (r + 2):(r + 21):18, 1:129],
                                             wqh[:, :, 3 * C:4 * C],
                                             start=False, stop=False, perf_mode=DR)
                            # hi: plain single (dy=+1, dx=-1)
                            nc.tensor.matmul(psq, yhi[:, r + 2, 0:128],
                                             wqh[:, 0, 4 * C:5 * C],
                                             start=False, stop=False)
                            # lo: quad DR (block0), half DR (block1)
                            nc.tensor.matmul(psq, ylo[:, r:r + 2, 0:128],
                                             wql[:, :, 0:C],
                                             start=False, stop=False, perf_mode=DR)
                            nc.tensor.matmul(psq, ylo[:, r:r + 2, 2:130],
                                             wql[:, :, C:2 * C],
                                             start=False, stop=False, perf_mode=DR)
                            # lo: plain (dy=+1 row, 2 taps via dup)
                            nc.tensor.matmul(psq, ylo[:, r + 2, 0:128],
                                             wql[:, 0, 2 * C:3 * C],
                                             start=False, stop=False)
                            # lo: plain 64p (dy=+1, dx=+1)
                            nc.tensor.matmul(psq, ylo[64:128, r + 2, 1:129],
                                             wql[64:128, 0, 3 * C:4 * C],
                                             start=False, stop=True)
                            qsb = qsb2[:, (r - r0) * C:(r - r0) * C + C]
                            cur_q.append(qsb)
                            if r == r0 + 1:
                                nc.scalar.copy(qsb2[:], psq2[:])
                            if s > 0 and "attn" not in ablate:
                                attn_mms(prev_q[r], r, prev_kt,
                                         f=(s == 1 and r == 0), l=False)
                        prev_q, prev_kt = cur_q, kt

                    # drain attn for the last stripe
                    for r in range(SR):
                        if "attn" not in ablate:
                            attn_mms(prev_q[r], r, prev_kt, f=False, l=(r == SR - 1))
                    if "attn" in ablate:
                        nc.tensor.matmul(gq_hi, prev_q[0][:, 0:128], prev_q[0][:, 0:128], start=True, stop=True)
                        nc.tensor.matmul(gq_lo, prev_q[0][:, 128:192], prev_q[0][:, 128:192], start=True, stop=True)
                        nc.tensor.matmul(raw01, prev_q[0][:, 0:96], prev_kt[:, 0, 0:96], start=True, stop=True)
                        nc.tensor.matmul(raw23, prev_q[0][:, 96:192], prev_kt[:, 0, 96:192], start=True, stop=True)

                # ================= phase 2: softmax + MT =================
                with ExitStack() as sctx2:
                    s2 = sctx2.enter_context(tc.tile_pool(name="s2", bufs=1))
                    p2 = sctx2.enter_context(tc.tile_pool(name="p2", bufs=1, space="PSUM"))

                    # ssq_q from gram diagonals
                    scr2 = s2.tile([128, 128], F32)
                    ssqqA = s2.tile([128, 1], F32)
                    nc.vector.scalar_tensor_tensor(scr2[:], gq_hi, 1.0, identF[:],
                                                   AL.mult, AL.mult, accum_out=ssqqA[:])
                    scr2b = s2.tile([64, 64], F32)
                    ssqqB = s2.tile([64, 1], F32)
                    nc.vector.scalar_tensor_tensor(scr2b[:], gq_lo, 1.0, identF[0:64, 0:64],
                                                   AL.mult, AL.mult, accum_out=ssqqB[:])
                    # ssq_k totals
                    sskA = s2.tile([128, 1], F32)
                    nc.vector.reduce_sum(sskA[:], ssqA[:], axis=mybir.AxisListType.X)
                    sskB = s2.tile([64, 1], F32)
                    nc.vector.reduce_sum(sskB[:], ssqB[:], axis=mybir.AxisListType.X)

                    def rsqrt(dst, src):
                        nc.vector.reciprocal(dst, src)
                        nc.scalar.activation(dst, dst, AF.Sqrt)

                    rqA = s2.tile([128, 1], F32, name="rqA")
                    rsqrt(rqA[:], ssqqA[:])
                    rqB = s2.tile([64, 1], F32, name="rqB")
                    rsqrt(rqB[:], ssqqB[:])
                    rkA = s2.tile([128, 1], F32, name="rkA")
                    rsqrt(rkA[:], sskA[:])
                    rkB = s2.tile([64, 1], F32, name="rkB")
                    rsqrt(rkB[:], sskB[:])
                    # rq * tau
                    nc.vector.tensor_mul(rqA[:], rqA[:], tauA[:])
                    nc.vector.tensor_mul(rqB[:], rqB[:], tauB[:])

                    # rk rows [1, 96] then broadcast [96, 96] via K=1 matmul with ones
                    nc.sync.dma_start(rscr[0:1, 0:128].rearrange("a b -> b a"), rkA[:])
                    nc.sync.dma_start(rscr[0:1, 128:192].rearrange("a b -> b a"), rkB[:])
                    rkrow = s2.tile([1, 192], F32)
                    nc.sync.dma_start(rkrow[:], rscr)
                    rkrow_b = s2.tile([1, 192], BF16)
                    nc.vector.tensor_copy(rkrow_b[:], rkrow[:])
                    ones1 = s2.tile([1, 96], BF16)
                    nc.vector.memset(ones1[:], 1.0)
                    rkb01p = p2.tile([96, 96], F32)
                    nc.tensor.matmul(rkb01p[:], ones1[:], rkrow_b[0:1, 0:96], start=True, stop=True)
                    rkb23p = p2.tile([96, 96], F32)
                    nc.tensor.matmul(rkb23p[:], ones1[:], rkrow_b[0:1, 96:192], start=True, stop=True)

                    # logits = raw * (rq*tau) * rk
                    l01 = s2.tile([96, 96], F32)
                    nc.scalar.activation(l01[:], raw01, AF.Copy, scale=rqA[0:96, :])
                    nc.vector.tensor_mul(l01[:], l01[:], rkb01p[:])
                    l23 = s2.tile([96, 96], F32)
                    rq23 = s2.tile([96, 1], F32)
                    nc.sync.dma_start(rq23[0:32, :], rqA[96:128, :])
                    nc.sync.dma_start(rq23[32:96, :], rqB[:])
                    nc.scalar.activation(l23[:], raw23, AF.Copy, scale=rq23[:])
                    nc.vector.tensor_mul(l23[:], l23[:], rkb23p[:])

                    # softmax per head-pair with additive block mask -> blockdiag bd (bf16)
                    msk = s2.tile([96, 96], F32)
                    nc.sync.dma_start(msk[:], bmask)
                    bd01 = s2.tile([96, 96], BF16)
                    bd23 = s2.tile([96, 96], BF16)
                    for hb, (lt, bd) in enumerate(((l01, bd01), (l23, bd23))):
                        nc.vector.tensor_add(lt[:], lt[:], msk[:])
                        mx = s2.tile([96, 1], F32, tag=f"mx{hb}", name=f"mx{hb}")
                        nc.vector.reduce_max(mx[:], lt[:], axis=mybir.AxisListType.X)
                        nc.vector.tensor_scalar_mul(mx[:], mx[:], -1.0)
                        ex = s2.tile([96, 96], F32, tag=f"ex{hb}", name=f"ex{hb}")
                        rs = s2.tile([96, 1], F32, tag=f"rs{hb}", name=f"rs{hb}")
                        nc.scalar.activation(ex[:], lt[:], AF.Exp, bias=mx[:], accum_out=rs[:])
                        nc.vector.reciprocal(rs[:], rs[:])
                        nc.vector.tensor_scalar_mul(bd[:], ex[:], rs[:])

                    # MT[d, o] = sum_c attn[c, d] * projT[c, o]
                    mt_hi_p = p2.tile([96, C], F32)
                    nc.tensor.matmul(mt_hi_p[:], bd01[:], prA[:], start=True, stop=True)
                    mt_lo_p = p2.tile([96, C], F32)
                    nc.tensor.matmul(mt_lo_p[:], bd23[:], prB[:], start=True, stop=True)
                    mt_hi = wp.tile([96, C], BF16)
                    nc.scalar.copy(mt_hi[:], mt_hi_p[:])
                    mt_lo = wp.tile([96, C], BF16)
                    nc.scalar.copy(mt_lo[:], mt_lo_p[:])

                # ================= phase 3: out = MT.T @ v, streamed =================
                with ExitStack() as sctx3:
                    s3 = sctx3.enter_context(tc.tile_pool(name="s3", bufs=3))
                    p3 = sctx3.enter_context(tc.tile_pool(name="p3", bufs=2, space="PSUM"))
                    for g in range(NCK // 4):
                        gsl = slice(2048 * g, 2048 * g + 2048)
                        vhi = s3.tile([96, 2048], BF16, tag="vhi")
                        nc.sync.dma_start(vhi[:], vscr[0:96, gsl])
                        vlo = s3.tile([96, 2048], BF16, tag="vlo")
                        nc.sync.dma_start(vlo[:], vscr[96:192, gsl])
                        o1g = s3.tile([128, 2048], BF16, tag="o1")
                        o2g = s3.tile([64, 2048], BF16, tag="o2")
                        for jj in range(4):
                            vsl = slice(512 * jj, 512 * jj + 512)
                            f1 = p3.tile([128, 512], F32, tag="f1")
                            nc.tensor.matmul(f1[:], mt_hi[:, 0:128], vhi[:, vsl], start=True, stop=False)
                            nc.tensor.matmul(f1[:], mt_lo[:, 0:128], vlo[:, vsl], start=False, stop=True)
                            f2 = p3.tile([64, 512], F32, tag="f2")
                            nc.tensor.matmul(f2[:], mt_hi[:, 128:192], vhi[:, vsl], start=True, stop=False)
                            nc.tensor.matmul(f2[:], mt_lo[:, 128:192], vlo[:, vsl], start=False, stop=True)
                            nc.vector.tensor_copy(o1g[:, vsl], f1[:])
                            nc.scalar.copy(o2g[:, vsl], f2[:])
                        nc.scalar.dma_start(out[0:128, gsl], o1g[:])
                        nc.sync.dma_start(out[128:192, gsl], o2g[:])

    nc.compile()
    return nc


# ======================= harness entry point =======================
B = 8
H = 128
_NC = None


def _get_nc():
    global _NC
    if _NC is None:
        _NC = build(H=H)
    return _NC


def _make_in_maps(inputs, H=H):
    import ml_dtypes
    xf = np.ascontiguousarray(inputs["x"], np.float32)
    x = np.asarray(xf, dtype=ml_dtypes.bfloat16)
    x8full = np.asarray(xf, dtype=ml_dtypes.float8_e4m3)
    y = np.asarray(np.ascontiguousarray(inputs["y"], np.float32), dtype=ml_dtypes.float8_e4m3)
    prep = host_prep(inputs["kv_w"], inputs["kv_dw_w"], inputs["q_w"],
                     inputs["q_dw_w"], inputs["proj_w"], inputs["temperature"])
    maps = []
    for b in range(x.shape[0]):
        m = {"x": x[b].reshape(C, H * W), "y": y[b].reshape(C, H * W),
             "x8": x8full[b].reshape(C, H * W)}
        m.update(prep)
        maps.append(m)
    return maps


def _run(inputs, trace=False, trace_kwargs=None):
    from concourse.bass_utils import run_bass_kernel_spmd
    nc = _get_nc()
    res = run_bass_kernel_spmd(nc, _make_in_maps(inputs), core_ids=list(range(B)),
                               trace=trace, trace_kwargs=trace_kwargs or {})
    out = np.stack([np.asarray(res.results[b]["out"]).astype(np.float32).reshape(C, H, W)
                    for b in range(B)])
    return out, res


def kernel(**inputs) -> np.ndarray:
    out, _ = _run(inputs, trace=False)
    return out


# revision 5
# speedup vs baseline: 1.2938x; 1.2407x over previous
import math
from contextlib import ExitStack

import numpy as np

import concourse.bass as bass
import concourse.tile as tile
from concourse import bacc, mybir
from concourse.masks import make_identity

F32 = mybir.dt.float32
BF16 = mybir.dt.bfloat16
FP8 = mybir.dt.float8e4
AL = mybir.AluOpType
AF = mybir.ActivationFunctionType
DR = mybir.MatmulPerfMode.DoubleRow

C = 192          # channels
HEADS = 4
CH = C // HEADS  # 48
W = 128          # image width
SR = 16          # rows per stripe
PW = W + 2       # padded width (bf16 kvp tiles)
PWQ = 144        # padded width for fp8 q staging (16B-aligned pair strides)
PR = SR + 2      # padded rows per stripe

S_WEFF = 256.0   # fp8 scale for q-conv weights (cancels in l2-norm)
S_KV = 64.0      # (unused: kv conv stays bf16 -- v cannot tolerate fp8)

# tap index t = (dy+1)*3 + (dx+1)
T = {(dy, dx): (dy + 1) * 3 + (dx + 1) for dy in (-1, 0, 1) for dx in (-1, 0, 1)}


def host_prep(kv_w, kv_dw_w, q_w, q_dw_w, proj_w, temperature):
    """Host-side weight transforms (all tiny). Returns dict of extra device inputs."""
    import ml_dtypes
    f8 = lambda a: np.asarray(a, dtype=ml_dtypes.float8_e4m3)
    bf = lambda a: np.asarray(a, dtype=ml_dtypes.bfloat16)

    kv_w = kv_w.astype(np.float64)
    q_w = q_w.astype(np.float64)
    q_dw_w = q_dw_w.astype(np.float64)
    proj_w = proj_w.astype(np.float64)

    # fused dense conv: W_eff[o, j, tap] = sum_i q_dw_w[o,i,tap] * q_w[i,j]
    weff = np.einsum("oiyx,ij->ojyx", q_dw_w, q_w[:, :, 0, 0]).reshape(C, C, 9)
    weff = weff * S_WEFF  # [o, j, tap]

    # ---- q-conv DR moving weights ----
    # hi chunk (contract ch j in 0:128), 5 MM blocks:
    #   MM0..2: row-pair (dy=-1,dy=0) at dx=-1,0,+1 -> slots (T[-1,dx], T[0,dx])
    #   MM3: region pair -> slots (T[1,0], T[1,1])
    #   MM4: plain single -> T[1,-1]
    wqh = np.zeros((128, 2, 5 * C), np.float64)
    for b, dx in enumerate((-1, 0, 1)):
        wqh[:, 0, b * C:(b + 1) * C] = weff[:, 0:128, T[(-1, dx)]].T
        wqh[:, 1, b * C:(b + 1) * C] = weff[:, 0:128, T[(0, dx)]].T
    wqh[:, 0, 3 * C:4 * C] = weff[:, 0:128, T[(1, 0)]].T
    wqh[:, 1, 3 * C:4 * C] = weff[:, 0:128, T[(1, 1)]].T
    wqh[:, 0, 4 * C:5 * C] = weff[:, 0:128, T[(1, -1)]].T

    # lo chunk (contract ch j in 128:192) via partition-dup tile:
    #   partitions p<64 = ch 128+p (cols at +1), p>=64 = ch 128+(p-64) (cols at 0, dx+1)
    #   L0 (block0, quad): p<64 slots (T[-1,-1], T[0,-1]); p>=64 slots (T[-1,0], T[0,0])
    #   L1 (block1): p<64 slots (T[-1,1], T[0,1]); p>=64 ZERO
    #   L2 (block2, plain): p<64 T[1,-1]; p>=64 T[1,0]
    #   L3 (block3, plain, p>=64 only, 64 contract): T[1,1]
    wql = np.zeros((128, 2, 4 * C), np.float64)
    lo = weff[:, 128:192, :]  # [o, 64, 9]
    wql[0:64, 0, 0:C] = lo[:, :, T[(-1, -1)]].T
    wql[0:64, 1, 0:C] = lo[:, :, T[(0, -1)]].T
    wql[64:128, 0, 0:C] = lo[:, :, T[(-1, 0)]].T
    wql[64:128, 1, 0:C] = lo[:, :, T[(0, 0)]].T
    wql[0:64, 0, C:2 * C] = lo[:, :, T[(-1, 1)]].T
    wql[0:64, 1, C:2 * C] = lo[:, :, T[(0, 1)]].T
    wql[0:64, 0, 2 * C:3 * C] = lo[:, :, T[(1, -1)]].T
    wql[64:128, 0, 2 * C:3 * C] = lo[:, :, T[(1, 0)]].T
    wql[0:64, 0, 3 * C:4 * C] = lo[:, :, T[(1, 1)]].T  # used as [64,192] block

    # kv 1x1: chunk m=0 (pure k) in fp8-DR; chunks 1,2 bf16 (v cannot tolerate fp8)
    wkvT = kv_w[:, :, 0, 0].T.copy()  # [192, 384]
    wkv8 = np.zeros((96, 2, 128), np.float64)  # contract slot (p,i) = in-ch p+96i
    wkv8[:, 0, :] = (kv_w[0:128, 0:96, 0, 0] * S_KV).T
    wkv8[:, 1, :] = (kv_w[0:128, 96:192, 0, 0] * S_KV).T

    projT = proj_w[:, :, 0, 0].T.copy()  # [c, o]

    dws = kv_dw_w[:, 0].reshape(2 * C, 9).copy()  # [384, 9]
    dwdiag = np.zeros((128, 2 * 9 * 128), np.float64)
    for mc in range(2):          # kv1 chunks 1 and 2 (channels 128..383)
        for t in range(9):
            col = (9 * mc + t) * 128
            dwdiag[np.arange(128), col + np.arange(128)] = dws[128 * (mc + 1):128 * (mc + 2), t]
    tau = np.repeat(np.asarray(temperature, np.float64).reshape(HEADS), CH) * math.log(CH)
    m = np.full((96, 96), -1e9, np.float32)
    m[0:48, 0:48] = 0.0
    m[48:96, 48:96] = 0.0
    return {
        "bmask": m,
        "wqh": f8(wqh.reshape(128, 2 * 5 * C)),
        "wql": f8(wql.reshape(128, 2 * 4 * C)),
        "wkvT": bf(wkvT),
        "wkv8": f8(wkv8.reshape(96, 2 * 128)),
        "projT": bf(projT),
        "dws": dws.astype(np.float32),
        "dwdiag": bf(dwdiag),
        "tau": tau.reshape(C, 1).astype(np.float32),
    }


def build(H=128, debug=False, dw_on_pe=0, reps=1, ablate=(), bufs=None):
    """Build + compile the per-core program. H = image height (rows)."""
    HW = H * W
    NS = H // SR  # stripes
    NCK = HW // 512  # output chunks

    nc = bacc.Bacc("TRN2", target_bir_lowering=False, debug=debug,
                   enable_asserts=False, num_devices=1)
    x = nc.dram_tensor("x", [C, HW], BF16, kind="ExternalInput").ap()
    x8 = nc.dram_tensor("x8", [C, HW], FP8, kind="ExternalInput").ap()
    wkv8_d = nc.dram_tensor("wkv8", [96, 2 * 128], FP8, kind="ExternalInput").ap()
    y = nc.dram_tensor("y", [C, HW], FP8, kind="ExternalInput").ap()
    wqh_d = nc.dram_tensor("wqh", [128, 2 * 5 * C], FP8, kind="ExternalInput").ap()
    wql_d = nc.dram_tensor("wql", [128, 2 * 4 * C], FP8, kind="ExternalInput").ap()
    wkvT = nc.dram_tensor("wkvT", [C, 2 * C], BF16, kind="ExternalInput").ap()
    projT = nc.dram_tensor("projT", [C, C], BF16, kind="ExternalInput").ap()
    dws = nc.dram_tensor("dws", [2 * C, 9], F32, kind="ExternalInput").ap()
    dwdiag = nc.dram_tensor("dwdiag", [128, 2 * 9 * 128], BF16, kind="ExternalInput").ap()
    tau = nc.dram_tensor("tau", [C, 1], F32, kind="ExternalInput").ap()
    out = nc.dram_tensor("out", [C, HW], BF16, kind="ExternalOutput").ap()
    vscr = nc.dram_tensor("vscr", [C, HW], BF16, kind="Internal").ap()
    rscr = nc.dram_tensor("rscr", [1, C], F32, kind="Internal").ap()
    bmask = nc.dram_tensor("bmask", [96, 96], F32, kind="ExternalInput").ap()

    with tile.TileContext(nc) as tc:
        with ExitStack() as ctx:
            wp = ctx.enter_context(tc.tile_pool(name="wp", bufs=1))       # persistent sbuf
            pp = ctx.enter_context(tc.tile_pool(name="pp", bufs=1, space="PSUM"))  # persistent psum

            # ---- weights straight to sbuf ----
            wqh = wp.tile([128, 2, 5 * C], FP8, name="wqh_t")
            nc.sync.dma_start(wqh[:].rearrange("p a b -> p (a b)"), wqh_d)
            wql = wp.tile([128, 2, 4 * C], FP8, name="wql_t")
            nc.sync.dma_start(wql[:].rearrange("p a b -> p (a b)"), wql_d)
            wkv8_t = wp.tile([96, 2, 128], FP8, name="wkv8_t")
            nc.sync.dma_start(wkv8_t[:].rearrange("p a b -> p (a b)"), wkv8_d)
            wkv_hi = wp.tile([128, 2 * C], BF16, name="wkv_hi")
            nc.sync.dma_start(wkv_hi[:], wkvT[0:128, :])
            wkv_lo = wp.tile([64, 2 * C], BF16, name="wkv_lo")
            nc.sync.dma_start(wkv_lo[:], wkvT[128:192, :])
            ddg = wp.tile([128, 2 * 9 * 128], BF16, name="ddg")
            nc.sync.dma_start(ddg[:], dwdiag)
            prA = wp.tile([96, C], BF16, name="prA")
            nc.sync.dma_start(prA[:], projT[0:96, :])
            prB = wp.tile([96, C], BF16, name="prB")
            nc.sync.dma_start(prB[:], projT[96:192, :])
            dws_t = wp.tile([128, 9 * 3], F32)  # 3 chunks side by side: [:,9m+t]
            for m in range(3):
                nc.sync.dma_start(dws_t[0:128, 9 * m:9 * m + 9], dws[128 * m:128 * m + 128, :])
            tauA = wp.tile([128, 1], F32)
            nc.sync.dma_start(tauA[:], tau[0:128, :])
            tauB = wp.tile([64, 1], F32)
            nc.sync.dma_start(tauB[:], tau[128:192, :])
            identF = wp.tile([128, 128], F32)
            make_identity(nc, identF[:])

            # preload activation tables used later (Copy/Square/Sqrt/Exp)
            actw = wp.tile([128, 1], F32)
            nc.vector.memset(actw[:], 1.0)
            for fn in (AF.Copy, AF.Square, AF.Sqrt, AF.Exp):
                nc.scalar.activation(actw[:], actw[:], fn)

            # ssq accumulator slots (per stripe), fp32
            ssqA = wp.tile([128, NS], F32)   # k ch 0..127
            ssqB = wp.tile([64, NS], F32)    # k ch 128..191

            # persistent psum: attn raw blocks + q gram packed into ONE bank
            accb = pp.tile([128, 512], F32, name="accb")
            raw01 = accb[0:96, 0:96]
            raw23 = accb[0:96, 96:192]
            gq_hi = accb[:, 192:320]
            gq_lo = accb[0:64, 320:384]

            taps = [(dy, dx) for dy in (-1, 0, 1) for dx in (-1, 0, 1)]

            for rep in range(reps):
                with ExitStack() as sctx:
                    bu = bufs if bufs is not None else {"qp": 33, "sp": 3, "pk": 3, "pq": 3}
                    sp = sctx.enter_context(tc.tile_pool(name="sp", bufs=bu.get("sp", 2)))      # stripe transients
                    qp = sctx.enter_context(tc.tile_pool(name="qp", bufs=bu.get("qp", 34)))
                    kvpool = sctx.enter_context(tc.tile_pool(name="kvp", bufs=bu.get("kvp", 3)))
                    pk = sctx.enter_context(tc.tile_pool(name="pk", bufs=bu.get("pk", 2), space="PSUM"))
                    pq = sctx.enter_context(tc.tile_pool(name="pq", bufs=bu.get("pq", 2), space="PSUM"))

                    for s in range(NS):
                        r_lo = SR * s - 1           # first (halo) image row
                        n_lo = r_lo * W
                        t_a = 1 if s == 0 else 0          # first valid tile row
                        t_b = PR - 1 if s == NS - 1 else PR  # one past last valid tile row

                        # ---- y staging: fp8 padded tiles, direct DMA ----
                        # yhi [128, 36, PWQ]: rows 0:18 region0 (y cols at +1),
                        #                     rows 18:36 region1 (y cols at +0)
                        yhi = sp.tile([128, 2 * PR, PWQ], FP8, tag="yhi", name="yhi")
                        # ylo [128, 18, PWQ]: p<64 ch128+p (cols at +1), p>=64 dup (cols at +0)
                        ylo = sp.tile([128, PR, PWQ], FP8, tag="ylo", name="ylo")
                        if s == 0:
                            nc.gpsimd.memset(yhi[:, 0:1, :], 0.0)
                            nc.gpsimd.memset(yhi[:, PR:PR + 1, :], 0.0)
                            nc.gpsimd.memset(ylo[:, 0:1, :], 0.0)
                        if s == NS - 1:
                            nc.gpsimd.memset(yhi[:, PR - 1:PR, :], 0.0)
                            nc.gpsimd.memset(yhi[:, 2 * PR - 1:2 * PR, :], 0.0)
                            nc.gpsimd.memset(ylo[:, PR - 1:PR, :], 0.0)
                        # pad columns
                        nc.gpsimd.memset(yhi[:, 0:PR, 0:1], 0.0)            # region0 left pad
                        nc.gpsimd.memset(yhi[:, 0:PR, 129:130], 0.0)        # region0 right pad
                        nc.gpsimd.memset(yhi[:, PR:2 * PR, 128:130], 0.0)   # region1 right pad
                        nc.gpsimd.memset(ylo[0:64, :, 0:1], 0.0)
                        nc.gpsimd.memset(ylo[0:64, :, 129:130], 0.0)
                        nc.gpsimd.memset(ylo[64:128, :, 128:130], 0.0)
                        nrows = t_b - t_a
                        src_off = n_lo + t_a * W
                        src_hi = bass.AP(y.tensor, src_off, [[HW, 128], [W, nrows], [1, W]])
                        src_lo = bass.AP(y.tensor, 128 * HW + src_off, [[HW, 64], [W, nrows], [1, W]])
                        if "ystage" not in ablate or s < 3:
                            nc.sync.dma_start(yhi[:, t_a:t_b, 1:1 + W], src_hi)
                            nc.scalar.dma_start(yhi[:, PR + t_a:PR + t_b, 0:W], src_hi)
                            nc.scalar.dma_start(ylo[0:64, t_a:t_b, 1:1 + W], src_lo)
                            nc.gpsimd.dma_start(ylo[64:128, t_a:t_b, 0:W], src_lo)

                        # ---- x staging: bf16 contiguous tiles ----
                        xbA = sp.tile([128, PR * W], BF16, tag="xbA", name="xbA")
                        xbB = sp.tile([64, PR * W], BF16, tag="xbB", name="xbB")
                        if s == 0:
                            nc.gpsimd.memset(xbA[:, 0:W], 0.0)
                            nc.gpsimd.memset(xbB[:, 0:W], 0.0)
                        if s == NS - 1:
                            nc.gpsimd.memset(xbA[:, (PR - 1) * W:], 0.0)
                            nc.gpsimd.memset(xbB[:, (PR - 1) * W:], 0.0)
                        nc.sync.dma_start(xbA[:, t_a * W:t_b * W],
                                          x[0:128, src_off:src_off + (t_b - t_a) * W])
                        nc.sync.dma_start(xbB[:, t_a * W:t_b * W],
                                          x[128:192, src_off:src_off + (t_b - t_a) * W])
                        x84 = sp.tile([96, 2, PR * W], FP8, tag="x84", name="x84")
                        if s == 0:
                            nc.gpsimd.memset(x84[:, :, 0:W], 0.0)
                        if s == NS - 1:
                            nc.gpsimd.memset(x84[:, :, (PR - 1) * W:], 0.0)
                        src_x0 = bass.AP(x8.tensor, src_off, [[HW, 96], [1, nrows * W]])
                        src_x1 = bass.AP(x8.tensor, 96 * HW + src_off, [[HW, 96], [1, nrows * W]])
                        nc.scalar.dma_start(x84[:, 0, t_a * W:t_b * W], src_x0)
                        nc.scalar.dma_start(x84[:, 1, t_a * W:t_b * W], src_x1)

                        # ---- kv 1x1 conv (fp8 DR) -> padded kvp chunks (bf16) ----
                        kvp = []
                        for m in range(3):
                            kvt = kvpool.tile([128, PR, PW], BF16, tag=f"kvp{m}")
                            nc.gpsimd.memset(kvt[:, :, 0:1], 0.0)
                            nc.gpsimd.memset(kvt[:, :, PW - 1:PW], 0.0)
                            kvp.append(kvt)
                            lhs_hi = wkv_hi[:, 128 * m:128 * m + 128]
                            lhs_lo = wkv_lo[:, 128 * m:128 * m + 128]
                            for j in range(0, PR * W, 512):
                                w_ = min(512, PR * W - j)
                                pst = pk.tile([128, 512], F32, tag="pkv", name="pkv")
                                ps = pst[:, 0:w_]
                                if m == 0:
                                    nc.tensor.matmul(ps, wkv8_t[:], x84[:, :, j:j + w_],
                                                     start=True, stop=True, perf_mode=DR)
                                else:
                                    nc.tensor.matmul(ps, lhs_hi, xbA[:, j:j + w_], start=True, stop=False)
                                    nc.tensor.matmul(ps, lhs_lo, xbB[:, j:j + w_], start=False, stop=True)
                                if "kvcopy" not in ablate or s < 3:
                                    nc.scalar.copy(
                                        kvt[:, j // W:j // W + w_ // W, 1:1 + W],
                                        ps.rearrange("p (a b) -> p a b", b=W))

                        # ---- depthwise 3x3 (DVE, optionally some chunks on PE) ----
                        kA = sp.tile([128, SR, W], BF16, tag="kA")
                        kvmid = sp.tile([128, SR, W], BF16, tag="kvmid")
                        vB = sp.tile([128, SR, W], BF16, tag="vB")
                        douts = [kA, kvmid, vB]
                        if "dw" in ablate:
                            for dsta in douts:
                                nc.vector.memset(dsta[:], 0.01)
                        for m in range(0 if "dw" in ablate else (3 - dw_on_pe)):
                            dst = douts[m]
                            for ti, (dy, dx) in enumerate(taps):
                                sc = dws_t[:, 9 * m + ti:9 * m + ti + 1]
                                src = kvp[m][:, 1 + dy:1 + SR + dy, 1 + dx:1 + dx + W]
                                d = dst[:, :, :]
                                if ti == 0:
                                    nc.vector.tensor_scalar_mul(d, src, sc)
                                else:
                                    tmp = sp.tile([128, SR, W], BF16, tag=f"dwtmp", name="dwtmp")
                                    nc.vector.tensor_scalar_mul(tmp[:], src, sc)
                                    nc.vector.tensor_add(d, d, tmp[:])
                        for m in range(3 - dw_on_pe, 3):
                            dst = douts[m]
                            for jr in range(0, SR, 4):
                                pdw = pk.tile([128, 512], F32, tag="pkv", name="pdw")
                                for ti, (dy, dx) in enumerate(taps):
                                    lhs = ddg[:, (9 * (m - 1) + ti) * 128:(9 * (m - 1) + ti + 1) * 128]
                                    rhs = kvp[m][:, 1 + jr + dy:1 + jr + dy + 4, 1 + dx:1 + dx + W]
                                    nc.tensor.matmul(pdw[:], lhs, rhs, start=(ti == 0), stop=(ti == 8))
                                nc.vector.tensor_copy(
                                    dst[:, jr:jr + 4, :],
                                    pdw[:].rearrange("p (a b) -> p a b", b=W))

                        # ---- ssq_k ----
                        scr = sp.tile([128, SR, W], BF16, tag="dwtmp", name="scr")
                        if "ssq" not in ablate or True:
                            nc.scalar.activation(scr[:], kA[:], AF.Square, accum_out=ssqA[:, s:s + 1])
                            nc.scalar.activation(scr[0:64], kvmid[0:64], AF.Square, accum_out=ssqB[:, s:s + 1])
                        elif s == 0:
                            nc.vector.memset(ssqA[:], 1.0)
                            nc.vector.memset(ssqB[:], 1.0)

                        # ---- v spill ----
                        nsl = slice(SR * s * W, SR * s * W + SR * W)
                        nc.sync.dma_start(vscr[0:64, nsl], kvmid[64:128].rearrange("p a b -> p (a b)"))
                        nc.sync.dma_start(vscr[64:192, nsl], vB[:].rearrange("p a b -> p (a b)"))

                        # ---- k transpose: kt[p, r, c] = k[c, r*W + p] ----
                        kt = sp.tile([128, SR, C], BF16, tag="kt")
                        nc.sync.dma_start_transpose(kt[:, :, 0:128], kA[:].rearrange("p a b -> p (a b)"))
                        nc.sync.dma_start_transpose(kt[:, :, 128:192], kvmid[0:64].rearrange("p a b -> p (a b)"))

                        # ---- q conv (fp8 DR) + attn/gram accumulation ----
                        def attn_mms(qsb, r, kt_, f, l):
                            nc.tensor.matmul(gq_hi, qsb[:, 0:128], qsb[:, 0:128], start=f, stop=l)
                            nc.tensor.matmul(gq_lo, qsb[:, 128:192], qsb[:, 128:192], start=f, stop=l)
                            nc.tensor.matmul(raw01, qsb[:, 0:96], kt_[:, r, 0:96], start=f, stop=l)
                            nc.tensor.matmul(raw23, qsb[:, 96:192], kt_[:, r, 96:192], start=f, stop=l)

                        cur_q = []
                        for r0 in range(0, SR, 2):
                          psq2 = pq.tile([128, 2 * C], F32, tag="psq", name="psq2")
                          qsb2 = qp.tile([128, 2 * C], BF16, tag="qsb", name="qsb2")
                          for r in (r0, r0 + 1):
                            psq = psq2[:, (r - r0) * C:(r - r0) * C + C]
                            if "qconv" in ablate:
                                nc.tensor.matmul(psq, yhi[:, r:r + 2, 0:128],
                                                 wqh[:, :, 0:C],
                                                 start=True, stop=True, perf_mode=DR)
                                qsb = qsb2[:, (r - r0) * C:(r - r0) * C + C]
                                cur_q.append(qsb)
                                if r == r0 + 1:
                                    nc.scalar.copy(qsb2[:], psq2[:])
                                if s > 0:
                                    attn_mms(prev_q[r], r, prev_kt,
                                             f=(s == 1 and r == 0), l=False)
                                continue
                            # hi: 3 row-pair DR MMs (dx=-1,0,+1)
                            for b in range(3):
                                nc.tensor.matmul(psq, yhi[:, r:r + 2, b:b + 128],
                                                 wqh[:, :, b * C:(b + 1) * C],
                                                 start=(b == 0), stop=False, perf_mode=DR)
                            # hi: region-pair (dy=+1: dx=0 & dx=+1)
                            nc.tensor.matmul(psq, yhi[:, (r + 2):(r + 21):18, 1:129],
                                             wqh[:, :, 3 * C:4 * C],
                                             start=False, stop=False, perf_mode=DR)
                            # hi: plain single (dy=+1, dx=-1)
                            nc.tensor.matmul(psq, yhi[:, r + 2, 0:128],
                                             wqh[:, 0, 4 * C:5 * C],
                                             start=False, stop=False)
                            # lo: quad DR (block0), half DR (block1)
                            nc.tensor.matmul(psq, ylo[:, r:r + 2, 0:128],
                                             wql[:, :, 0:C],
                                             start=False, stop=False, perf_mode=DR)
                            nc.tensor.matmul(psq, ylo[:, r:r + 2, 2:130],
                                             wql[:, :, C:2 * C],
                                             start=False, stop=False, perf_mode=DR)
                            # lo: plain (dy=+1 row, 2 taps via dup)
                            nc.tensor.matmul(psq, ylo[:, r + 2, 0:128],
                                             wql[:, 0, 2 * C:3 * C],
                                             start=False, stop=False)
                            # lo: plain 64p (dy=+1, dx=+1)
                            nc.tensor.matmul(psq, ylo[64:128, r + 2, 1:129],
                                             wql[64:128, 0, 3 * C:4 * C],
                                             start=False, stop=True)
                            qsb = qsb2[:, (r - r0) * C:(r - r0) * C + C]
                            cur_q.append(qsb)
                            if r == r0 + 1:
                                nc.scalar.copy(qsb2[:], psq2[:])
                            if s > 0 and "attn" not in ablate:
                                attn_mms(prev_q[r], r, prev_kt,
                                         f=(s == 1 and r == 0), l=False)
                        prev_q, prev_kt = cur_q, kt

                    # drain attn for the last stripe
                    for r in range(SR):
                        if "attn" not in ablate:
                            attn_mms(prev_q[r], r, prev_kt, f=False, l=(r == SR - 1))
                    if "attn" in ablate:
                        nc.tensor.matmul(gq_hi, prev_q[0][:, 0:128], prev_q[0][:, 0:128], start=True, stop=True)
                        nc.tensor.matmul(gq_lo, prev_q[0][:, 128:192], prev_q[0][:, 128:192], start=True, stop=True)
                        nc.tensor.matmul(raw01, prev_q[0][:, 0:96], prev_kt[:, 0, 0:96], start=True, stop=True)
                        nc.tensor.matmul(raw23, prev_q[0][:, 96:192], prev_kt[:, 0, 96:192], start=True, stop=True)

                # ================= phase 2: softmax + MT =================
                with ExitStack() as sctx2:
                    s2 = sctx2.enter_context(tc.tile_pool(name="s2", bufs=1))
                    p2 = sctx2.enter_context(tc.tile_pool(name="p2", bufs=1, space="PSUM"))

                    # ssq_q from gram diagonals
                    scr2 = s2.tile([128, 128], F32)
                    ssqqA = s2.tile([128, 1], F32)
                    nc.vector.scalar_tensor_tensor(scr2[:], gq_hi, 1.0, identF[:],
                                                   AL.mult, AL.mult, accum_out=ssqqA[:])
                    scr2b = s2.tile([64, 64], F32)
                    ssqqB = s2.tile([64, 1], F32)
                    nc.vector.scalar_tensor_tensor(scr2b[:], gq_lo, 1.0, identF[0:64, 0:64],
                                                   AL.mult, AL.mult, accum_out=ssqqB[:])
                    # ssq_k totals
                    sskA = s2.tile([128, 1], F32)
                    nc.vector.reduce_sum(sskA[:], ssqA[:], axis=mybir.AxisListType.X)
                    sskB = s2.tile([64, 1], F32)
                    nc.vector.reduce_sum(sskB[:], ssqB[:], axis=mybir.AxisListType.X)

                    def rsqrt(dst, src):
                        nc.vector.reciprocal(dst, src)
                        nc.scalar.activation(dst, dst, AF.Sqrt)

                    rqA = s2.tile([128, 1], F32, name="rqA")
                    rsqrt(rqA[:], ssqqA[:])
                    rqB = s2.tile([64, 1], F32, name="rqB")
                    rsqrt(rqB[:], ssqqB[:])
                    rkA = s2.tile([128, 1], F32, name="rkA")
                    rsqrt(rkA[:], sskA[:])
                    rkB = s2.tile([64, 1], F32, name="rkB")
                    rsqrt(rkB[:], sskB[:])
                    # rq * tau
                    nc.vector.tensor_mul(rqA[:], rqA[:], tauA[:])
                    nc.vector.tensor_mul(rqB[:], rqB[:], tauB[:])

                    # rk rows [1, 96] then broadcast [96, 96] via K=1 matmul with ones
                    nc.sync.dma_start(rscr[0:1, 0:128].rearrange("a b -> b a"), rkA[:])
                    nc.sync.dma_start(rscr[0:1, 128:192].rearrange("a b -> b a"), rkB[:])
                    rkrow = s2.tile([1, 192], F32)
                    nc.sync.dma_start(rkrow[:], rscr)
                    rkrow_b = s2.tile([1, 192], BF16)
                    nc.vector.tensor_copy(rkrow_b[:], rkrow[:])
                    ones1 = s2.tile([1, 96], BF16)
                    nc.vector.memset(ones1[:], 1.0)
                    rkb01p = p2.tile([96, 96], F32)
                    nc.tensor.matmul(rkb01p[:], ones1[:], rkrow_b[0:1, 0:96], start=True, stop=True)
                    rkb23p = p2.tile([96, 96], F32)
                    nc.tensor.matmul(rkb23p[:], ones1[:], rkrow_b[0:1, 96:192], start=True, stop=True)

                    # logits = raw * (rq*tau) * rk
                    l01 = s2.tile([96, 96], F32)
                    nc.scalar.activation(l01[:], raw01, AF.Copy, scale=rqA[0:96, :])
                    nc.vector.tensor_mul(l01[:], l01[:], rkb01p[:])
                    l23 = s2.tile([96, 96], F32)
                    rq23 = s2.tile([96, 1], F32)
                    nc.sync.dma_start(rq23[0:32, :], rqA[96:128, :])
                    nc.sync.dma_start(rq23[32:96, :], rqB[:])
                    nc.scalar.activation(l23[:], raw23, AF.Copy, scale=rq23[:])
                    nc.vector.tensor_mul(l23[:], l23[:], rkb23p[:])

                    # softmax per head-pair with additive block mask -> blockdiag bd (bf16)
                    msk = s2.tile([96, 96], F32)
                    nc.sync.dma_start(msk[:], bmask)
                    bd01 = s2.tile([96, 96], BF16)
                    bd23 = s2.tile([96, 96], BF16)
                    for hb, (lt, bd) in enumerate(((l01, bd01), (l23, bd23))):
                        nc.vector.tensor_add(lt[:], lt[:], msk[:])
                        mx = s2.tile([96, 1], F32, tag=f"mx{hb}", name=f"mx{hb}")
                        nc.vector.reduce_max(mx[:], lt[:], axis=mybir.AxisListType.X)
                        nc.vector.tensor_scalar_mul(mx[:], mx[:], -1.0)
                        ex = s2.tile([96, 96], F32, tag=f"ex{hb}", name=f"ex{hb}")
                        rs = s2.tile([96, 1], F32, tag=f"rs{hb}", name=f"rs{hb}")
                        nc.scalar.activation(ex[:], lt[:], AF.Exp, bias=mx[:], accum_out=rs[:])
                        nc.vector.reciprocal(rs[:], rs[:])
                        nc.vector.tensor_scalar_mul(bd[:], ex[:], rs[:])

                    # MT[d, o] = sum_c attn[c, d] * projT[c, o]
                    mt_hi_p = p2.tile([96, C], F32)
                    nc.tensor.matmul(mt_hi_p[:], bd01[:], prA[:], start=True, stop=True)
                    mt_lo_p = p2.tile([96, C], F32)
                    nc.tensor.matmul(mt_lo_p[:], bd23[:], prB[:], start=True, stop=True)
                    mt_hi = wp.tile([96, C], BF16)
                    nc.scalar.copy(mt_hi[:], mt_hi_p[:])
                    mt_lo = wp.tile([96, C], BF16)
                    nc.scalar.copy(mt_lo[:], mt_lo_p[:])

                # ================= phase 3: out = MT.T @ v, streamed =================
                with ExitStack() as sctx3:
                    s3 = sctx3.enter_context(tc.tile_pool(name="s3", bufs=3))
                    p3 = sctx3.enter_context(tc.tile_pool(name="p3", bufs=2, space="PSUM"))
                    for g in range(NCK // 4):
                        gsl = slice(2048 * g, 2048 * g + 2048)
                        vhi = s3.tile([96, 2048], BF16, tag="vhi")
                        nc.sync.dma_start(vhi[:], vscr[0:96, gsl])
                        vlo = s3.tile([96, 2048], BF16, tag="vlo")
                        nc.sync.dma_start(vlo[:], vscr[96:192, gsl])
                        o1g = s3.tile([128, 2048], BF16, tag="o1")
                        o2g = s3.tile([64, 2048], BF16, tag="o2")
                        for jj in range(4):
                            vsl = slice(512 * jj, 512 * jj + 512)
                            f1 = p3.tile([128, 512], F32, tag="f1")
                            nc.tensor.matmul(f1[:], mt_hi[:, 0:128], vhi[:, vsl], start=True, stop=False)
                            nc.tensor.matmul(f1[:], mt_lo[:, 0:128], vlo[:, vsl], start=False, stop=True)
                            f2 = p3.tile([64, 512], F32, tag="f2")
                            nc.tensor.matmul(f2[:], mt_hi[:, 128:192], vhi[:, vsl], start=True, stop=False)
                            nc.tensor.matmul(f2[:], mt_lo[:, 128:192], vlo[:, vsl], start=False, stop=True)
                            nc.vector.tensor_copy(o1g[:, vsl], f1[:])
                            nc.scalar.copy(o2g[:, vsl], f2[:])
                        nc.scalar.dma_start(out[0:128, gsl], o1g[:])
                        nc.sync.dma_start(out[128:192, gsl], o2g[:])

    nc.compile()
    return nc


# ======================= harness entry point =======================
B = 8
H = 128
_NC = None


def _get_nc():
    global _NC
    if _NC is None:
        _NC = build(H=H)
    return _NC


def _make_in_maps(inputs, H=H):
    import ml_dtypes
    xf = np.ascontiguousarray(inputs["x"], np.float32)
    x = np.asarray(xf, dtype=ml_dtypes.bfloat16)
    x8full = np.asarray(xf, dtype=ml_dtypes.float8_e4m3)
    y = np.asarray(np.ascontiguousarray(inputs["y"], np.float32), dtype=ml_dtypes.float8_e4m3)
    prep = host_prep(inputs["kv_w"], inputs["kv_dw_w"], inputs["q_w"],
                     inputs["q_dw_w"], inputs["proj_w"], inputs["temperature"])
    maps = []
    for b in range(x.shape[0]):
        m = {"x": x[b].reshape(C, H * W), "y": y[b].reshape(C, H * W),
             "x8": x8full[b].reshape(C, H * W)}
        m.update(prep)
        maps.append(m)
    return maps


def _run(inputs, trace=False, trace_kwargs=None):
    from concourse.bass_utils import run_bass_kernel_spmd
    nc = _get_nc()
    res = run_bass_kernel_spmd(nc, _make_in_maps(inputs), core_ids=list(range(B)),
                               trace=trace, trace_kwargs=trace_kwargs or {})
    out = np.stack([np.asarray(res.results[b]["out"]).astype(np.float32).reshape(C, H, W)
                    for b in range(B)])
    return out, res


def kernel(**inputs) -> np.ndarray:
    out, _ = _run(inputs, trace=False)
    return out


# revision 6
# speedup vs baseline: 1.7316x; 1.3384x over previous
import math
from contextlib import ExitStack

import numpy as np

import concourse.bass as bass
import concourse.tile as tile
from concourse import bacc, mybir
from concourse.masks import make_identity

F32 = mybir.dt.float32
BF16 = mybir.dt.bfloat16
FP8 = mybir.dt.float8e4
AL = mybir.AluOpType
AF = mybir.ActivationFunctionType
DR = mybir.MatmulPerfMode.DoubleRow

C = 192          # channels
HEADS = 4
CH = C // HEADS  # 48
W = 128          # image width
SR = 16          # rows per stripe
PW = W + 2       # padded width (bf16 kvp tiles)
PWQ = 144        # padded width for fp8 q staging (16B-aligned pair strides)
PR = SR + 2      # padded rows per stripe

S_WEFF = 256.0   # fp8 scale for q-conv weights (cancels in l2-norm)
S_KV = 64.0      # (unused: kv conv stays bf16 -- v cannot tolerate fp8)

# tap index t = (dy+1)*3 + (dx+1)
T = {(dy, dx): (dy + 1) * 3 + (dx + 1) for dy in (-1, 0, 1) for dx in (-1, 0, 1)}


def host_prep(kv_w, kv_dw_w, q_w, q_dw_w, proj_w, temperature):
    """Host-side weight transforms (all tiny). Returns dict of extra device inputs."""
    import ml_dtypes
    f8 = lambda a: np.asarray(a, dtype=ml_dtypes.float8_e4m3)
    bf = lambda a: np.asarray(a, dtype=ml_dtypes.bfloat16)

    kv_w = kv_w.astype(np.float64)
    q_w = q_w.astype(np.float64)
    q_dw_w = q_dw_w.astype(np.float64)
    proj_w = proj_w.astype(np.float64)

    # fused dense conv: W_eff[o, j, tap] = sum_i q_dw_w[o,i,tap] * q_w[i,j]
    weff = np.einsum("oiyx,ij->ojyx", q_dw_w, q_w[:, :, 0, 0]).reshape(C, C, 9)
    weff = weff * S_WEFF  # [o, j, tap]

    # ---- q-conv DR moving weights ----
    # hi chunk (contract ch j in 0:128), 5 MM blocks:
    #   MM0..2: row-pair (dy=-1,dy=0) at dx=-1,0,+1 -> slots (T[-1,dx], T[0,dx])
    #   MM3: region pair -> slots (T[1,0], T[1,1])
    #   MM4: plain single -> T[1,-1]
    wqh = np.zeros((128, 2, 5 * C), np.float64)
    for b, dx in enumerate((-1, 0, 1)):
        wqh[:, 0, b * C:(b + 1) * C] = weff[:, 0:128, T[(-1, dx)]].T
        wqh[:, 1, b * C:(b + 1) * C] = weff[:, 0:128, T[(0, dx)]].T
    wqh[:, 0, 3 * C:4 * C] = weff[:, 0:128, T[(1, 0)]].T
    wqh[:, 1, 3 * C:4 * C] = weff[:, 0:128, T[(1, 1)]].T
    wqh[:, 0, 4 * C:5 * C] = weff[:, 0:128, T[(1, -1)]].T

    # lo chunk (contract ch j in 128:192) via partition-dup tile:
    #   partitions p<64 = ch 128+p (cols at +1), p>=64 = ch 128+(p-64) (cols at 0, dx+1)
    #   L0 (block0, quad): p<64 slots (T[-1,-1], T[0,-1]); p>=64 slots (T[-1,0], T[0,0])
    #   L1 (block1): p<64 slots (T[-1,1], T[0,1]); p>=64 ZERO
    #   L2 (block2, plain): p<64 T[1,-1]; p>=64 T[1,0]
    #   L3 (block3, plain, p>=64 only, 64 contract): T[1,1]
    wql = np.zeros((128, 2, 4 * C), np.float64)
    lo = weff[:, 128:192, :]  # [o, 64, 9]
    wql[0:64, 0, 0:C] = lo[:, :, T[(-1, -1)]].T
    wql[0:64, 1, 0:C] = lo[:, :, T[(0, -1)]].T
    wql[64:128, 0, 0:C] = lo[:, :, T[(-1, 0)]].T
    wql[64:128, 1, 0:C] = lo[:, :, T[(0, 0)]].T
    wql[0:64, 0, C:2 * C] = lo[:, :, T[(-1, 1)]].T
    wql[0:64, 1, C:2 * C] = lo[:, :, T[(0, 1)]].T
    wql[0:64, 0, 2 * C:3 * C] = lo[:, :, T[(1, -1)]].T
    wql[64:128, 0, 2 * C:3 * C] = lo[:, :, T[(1, 0)]].T
    wql[0:64, 0, 3 * C:4 * C] = lo[:, :, T[(1, 1)]].T  # used as [64,192] block

    # kv 1x1: chunk m=0 (pure k) in fp8-DR; chunks 1,2 bf16 (v cannot tolerate fp8)
    wkvT = kv_w[:, :, 0, 0].T.copy()  # [192, 384]
    wkv8 = np.zeros((96, 2, 128), np.float64)  # contract slot (p,i) = in-ch p+96i
    wkv8[:, 0, :] = (kv_w[0:128, 0:96, 0, 0] * S_KV).T
    wkv8[:, 1, :] = (kv_w[0:128, 96:192, 0, 0] * S_KV).T

    projT = proj_w[:, :, 0, 0].T.copy()  # [c, o]

    dws = kv_dw_w[:, 0].reshape(2 * C, 9).copy()  # [384, 9]
    ddg0 = np.zeros((128, 9 * 128), np.float64)   # chunk 0 diag blocks (transposed dw)
    for t in range(9):
        ddg0[np.arange(128), t * 128 + np.arange(128)] = dws[0:128, t]
    dwdiag = np.zeros((128, 2 * 9 * 128), np.float64)
    for mc in range(2):          # kv1 chunks 1 and 2 (channels 128..383)
        for t in range(9):
            col = (9 * mc + t) * 128
            dwdiag[np.arange(128), col + np.arange(128)] = dws[128 * (mc + 1):128 * (mc + 2), t]
    tau = np.repeat(np.asarray(temperature, np.float64).reshape(HEADS), CH) * math.log(CH)
    m = np.full((96, 96), -1e9, np.float32)
    m[0:48, 0:48] = 0.0
    m[48:96, 48:96] = 0.0
    return {
        "bmask": m,
        "wqh": f8(wqh.reshape(128, 2 * 5 * C)),
        "wql": f8(wql.reshape(128, 2 * 4 * C)),
        "wkvT": bf(wkvT),
        "wkv8": f8(wkv8.reshape(96, 2 * 128)),
        "projT": bf(projT),
        "dws": dws.astype(np.float32),
        "dwdiag": bf(dwdiag),
        "ddg0": bf(ddg0),
        "tau": tau.reshape(C, 1).astype(np.float32),
    }


def build(H=128, debug=False, dw_on_pe=0, reps=1, ablate=(), bufs=None):
    """Build + compile the per-core program. H = image height (rows)."""
    HW = H * W
    NS = H // SR  # stripes
    NCK = HW // 512  # output chunks

    nc = bacc.Bacc("TRN2", target_bir_lowering=False, debug=debug,
                   enable_asserts=False, num_devices=1)
    x = nc.dram_tensor("x", [C, HW], BF16, kind="ExternalInput").ap()
    x8 = nc.dram_tensor("x8", [C, HW], FP8, kind="ExternalInput").ap()
    wkv8_d = nc.dram_tensor("wkv8", [96, 2 * 128], FP8, kind="ExternalInput").ap()
    y = nc.dram_tensor("y", [C, HW], FP8, kind="ExternalInput").ap()
    wqh_d = nc.dram_tensor("wqh", [128, 2 * 5 * C], FP8, kind="ExternalInput").ap()
    wql_d = nc.dram_tensor("wql", [128, 2 * 4 * C], FP8, kind="ExternalInput").ap()
    wkvT = nc.dram_tensor("wkvT", [C, 2 * C], BF16, kind="ExternalInput").ap()
    projT = nc.dram_tensor("projT", [C, C], BF16, kind="ExternalInput").ap()
    dws = nc.dram_tensor("dws", [2 * C, 9], F32, kind="ExternalInput").ap()
    dwdiag = nc.dram_tensor("dwdiag", [128, 2 * 9 * 128], BF16, kind="ExternalInput").ap()
    ddg0_d = nc.dram_tensor("ddg0", [128, 9 * 128], BF16, kind="ExternalInput").ap()
    tau = nc.dram_tensor("tau", [C, 1], F32, kind="ExternalInput").ap()
    out = nc.dram_tensor("out", [C, HW], BF16, kind="ExternalOutput").ap()
    vscr = nc.dram_tensor("vscr", [C, HW], BF16, kind="Internal").ap()
    rscr = nc.dram_tensor("rscr", [1, C], F32, kind="Internal").ap()
    bmask = nc.dram_tensor("bmask", [96, 96], F32, kind="ExternalInput").ap()

    with tile.TileContext(nc) as tc:
        with ExitStack() as ctx:
            wp = ctx.enter_context(tc.tile_pool(name="wp", bufs=1))       # persistent sbuf
            pp = ctx.enter_context(tc.tile_pool(name="pp", bufs=1, space="PSUM"))  # persistent psum

            # ---- weights straight to sbuf ----
            wqh = wp.tile([128, 2, 5 * C], FP8, name="wqh_t")
            nc.sync.dma_start(wqh[:].rearrange("p a b -> p (a b)"), wqh_d)
            wql = wp.tile([128, 2, 4 * C], FP8, name="wql_t")
            nc.sync.dma_start(wql[:].rearrange("p a b -> p (a b)"), wql_d)
            wkv8_t = wp.tile([96, 2, 128], FP8, name="wkv8_t")
            nc.sync.dma_start(wkv8_t[:].rearrange("p a b -> p (a b)"), wkv8_d)
            wkv_hi = wp.tile([128, 2 * C], BF16, name="wkv_hi")
            nc.sync.dma_start(wkv_hi[:], wkvT[0:128, :])
            wkv_lo = wp.tile([64, 2 * C], BF16, name="wkv_lo")
            nc.sync.dma_start(wkv_lo[:], wkvT[128:192, :])
            ddg = wp.tile([128, 2 * 9 * 128], BF16, name="ddg")
            nc.sync.dma_start(ddg[:], dwdiag)
            ddg0 = wp.tile([128, 9 * 128], BF16, name="ddg0")
            nc.sync.dma_start(ddg0[:], ddg0_d)
            prA = wp.tile([96, C], BF16, name="prA")
            nc.sync.dma_start(prA[:], projT[0:96, :])
            prB = wp.tile([96, C], BF16, name="prB")
            nc.sync.dma_start(prB[:], projT[96:192, :])
            dws_t = wp.tile([128, 9 * 3], F32)  # 3 chunks side by side: [:,9m+t]
            for m in range(3):
                nc.sync.dma_start(dws_t[0:128, 9 * m:9 * m + 9], dws[128 * m:128 * m + 128, :])
            tauA = wp.tile([128, 1], F32)
            nc.sync.dma_start(tauA[:], tau[0:128, :])
            tauB = wp.tile([64, 1], F32)
            nc.sync.dma_start(tauB[:], tau[128:192, :])
            identF = wp.tile([128, 128], F32)
            make_identity(nc, identF[:])

            # preload activation tables used later (Copy/Square/Sqrt/Exp)
            actw = wp.tile([128, 1], F32)
            nc.vector.memset(actw[:], 1.0)
            for fn in (AF.Copy, AF.Square, AF.Sqrt, AF.Exp):
                nc.scalar.activation(actw[:], actw[:], fn)

            # ssq accumulator slots (per stripe), fp32
            ssqA = wp.tile([128, NS], F32)   # k ch 0..127
            ssqB = wp.tile([64, NS], F32)    # k ch 128..191

            # persistent psum: attn raw blocks + q gram packed into ONE bank
            accb = pp.tile([128, 512], F32, name="accb")
            raw01 = accb[0:96, 0:96]
            raw23 = accb[0:96, 96:192]
            gq_hi = accb[:, 192:320]
            gq_lo = accb[0:64, 320:384]
            gk_hi = accb[:, 384:512]

            taps = [(dy, dx) for dy in (-1, 0, 1) for dx in (-1, 0, 1)]

            for rep in range(reps):
                with ExitStack() as sctx:
                    bu = bufs if bufs is not None else {"qp": 33, "sp": 3, "pk": 2, "pq": 2, "pt": 2}
                    sp = sctx.enter_context(tc.tile_pool(name="sp", bufs=bu.get("sp", 2)))      # stripe transients
                    qp = sctx.enter_context(tc.tile_pool(name="qp", bufs=bu.get("qp", 34)))
                    kvpool = sctx.enter_context(tc.tile_pool(name="kvp", bufs=bu.get("kvp", 3)))
                    pk = sctx.enter_context(tc.tile_pool(name="pk", bufs=bu.get("pk", 2), space="PSUM"))
                    pq = sctx.enter_context(tc.tile_pool(name="pq", bufs=bu.get("pq", 2), space="PSUM"))
                    pt = sctx.enter_context(tc.tile_pool(name="pt", bufs=bu.get("pt", 2), space="PSUM"))

                    for s in range(NS):
                        r_lo = SR * s - 1           # first (halo) image row
                        n_lo = r_lo * W
                        t_a = 1 if s == 0 else 0          # first valid tile row
                        t_b = PR - 1 if s == NS - 1 else PR  # one past last valid tile row

                        # ---- y staging: fp8 padded tiles, direct DMA ----
                        # yhi [128, 36, PWQ]: rows 0:18 region0 (y cols at +1),
                        #                     rows 18:36 region1 (y cols at +0)
                        yhi = sp.tile([128, 2 * PR, PWQ], FP8, tag="yhi", name="yhi")
                        # ylo [128, 18, PWQ]: p<64 ch128+p (cols at +1), p>=64 dup (cols at +0)
                        ylo = sp.tile([128, PR, PWQ], FP8, tag="ylo", name="ylo")
                        if s == 0:
                            nc.gpsimd.memset(yhi[:, 0:1, :], 0.0)
                            nc.gpsimd.memset(yhi[:, PR:PR + 1, :], 0.0)
                            nc.gpsimd.memset(ylo[:, 0:1, :], 0.0)
                        if s == NS - 1:
                            nc.gpsimd.memset(yhi[:, PR - 1:PR, :], 0.0)
                            nc.gpsimd.memset(yhi[:, 2 * PR - 1:2 * PR, :], 0.0)
                            nc.gpsimd.memset(ylo[:, PR - 1:PR, :], 0.0)
                        # pad columns
                        nc.gpsimd.memset(yhi[:, 0:PR, 0:1], 0.0)            # region0 left pad
                        nc.gpsimd.memset(yhi[:, 0:PR, 129:130], 0.0)        # region0 right pad
                        nc.gpsimd.memset(yhi[:, PR:2 * PR, 128:130], 0.0)   # region1 right pad
                        nc.gpsimd.memset(ylo[0:64, :, 0:1], 0.0)
                        nc.gpsimd.memset(ylo[0:64, :, 129:130], 0.0)
                        nc.gpsimd.memset(ylo[64:128, :, 128:130], 0.0)
                        nrows = t_b - t_a
                        src_off = n_lo + t_a * W
                        src_hi = bass.AP(y.tensor, src_off, [[HW, 128], [W, nrows], [1, W]])
                        src_lo = bass.AP(y.tensor, 128 * HW + src_off, [[HW, 64], [W, nrows], [1, W]])
                        if "ystage" not in ablate or s < 3:
                            nc.sync.dma_start(yhi[:, t_a:t_b, 1:1 + W], src_hi)
                            nc.scalar.dma_start(yhi[:, PR + t_a:PR + t_b, 0:W], src_hi)
                            nc.scalar.dma_start(ylo[0:64, t_a:t_b, 1:1 + W], src_lo)
                            nc.gpsimd.dma_start(ylo[64:128, t_a:t_b, 0:W], src_lo)

                        # ---- x staging: bf16 contiguous tiles ----
                        xbA = sp.tile([128, PR * W], BF16, tag="xbA", name="xbA")
                        xbB = sp.tile([64, PR * W], BF16, tag="xbB", name="xbB")
                        if s == 0:
                            nc.gpsimd.memset(xbA[:, 0:W], 0.0)
                            nc.gpsimd.memset(xbB[:, 0:W], 0.0)
                        if s == NS - 1:
                            nc.gpsimd.memset(xbA[:, (PR - 1) * W:], 0.0)
                            nc.gpsimd.memset(xbB[:, (PR - 1) * W:], 0.0)
                        nc.sync.dma_start(xbA[:, t_a * W:t_b * W],
                                          x[0:128, src_off:src_off + (t_b - t_a) * W])
                        nc.sync.dma_start(xbB[:, t_a * W:t_b * W],
                                          x[128:192, src_off:src_off + (t_b - t_a) * W])
                        x84 = sp.tile([96, 2, PR * W], FP8, tag="x84", name="x84")
                        if s == 0:
                            nc.gpsimd.memset(x84[:, :, 0:W], 0.0)
                        if s == NS - 1:
                            nc.gpsimd.memset(x84[:, :, (PR - 1) * W:], 0.0)
                        src_x0 = bass.AP(x8.tensor, src_off, [[HW, 96], [1, nrows * W]])
                        src_x1 = bass.AP(x8.tensor, 96 * HW + src_off, [[HW, 96], [1, nrows * W]])
                        nc.scalar.dma_start(x84[:, 0, t_a * W:t_b * W], src_x0)
                        nc.scalar.dma_start(x84[:, 1, t_a * W:t_b * W], src_x1)

                        # ---- kv 1x1 conv (fp8 DR) -> padded kvp chunks (bf16) ----
                        kvp = []
                        for m in range(3):
                            kvt = kvpool.tile([128, PR, PW], BF16, tag=f"kvp{m}")
                            nc.gpsimd.memset(kvt[:, :, 0:1], 0.0)
                            nc.gpsimd.memset(kvt[:, :, PW - 1:PW], 0.0)
                            kvp.append(kvt)
                            lhs_hi = wkv_hi[:, 128 * m:128 * m + 128]
                            lhs_lo = wkv_lo[:, 128 * m:128 * m + 128]
                            for j in range(0, PR * W, 512):
                                w_ = min(512, PR * W - j)
                                pst = pk.tile([128, 512], F32, tag="pkv", name="pkv")
                                ps = pst[:, 0:w_]
                                if m == 0:
                                    nc.tensor.matmul(ps, wkv8_t[:], x84[:, :, j:j + w_],
                                                     start=True, stop=True, perf_mode=DR)
                                else:
                                    nc.tensor.matmul(ps, lhs_hi, xbA[:, j:j + w_], start=True, stop=False)
                                    nc.tensor.matmul(ps, lhs_lo, xbB[:, j:j + w_], start=False, stop=True)
                                if "kvcopy" not in ablate or s < 3:
                                    nc.scalar.copy(
                                        kvt[:, j // W:j // W + w_ // W, 1:1 + W],
                                        ps.rearrange("p (a b) -> p a b", b=W))

                        # ---- depthwise 3x3 (DVE, optionally some chunks on PE) ----
                        kA = None  # k ch 0:128 handled by transposed PE DW
                        kvmid = sp.tile([128, SR, W], BF16, tag="kvmid")
                        vB = sp.tile([128, SR, W], BF16, tag="vB")
                        douts = [kA, kvmid, vB]
                        if "dw" in ablate:
                            for dsta in douts:
                                nc.vector.memset(dsta[:], 0.01)
                        for m in ([] if "dw" in ablate else range(1, 3)):
                            dst = douts[m]
                            for ti, (dy, dx) in enumerate(taps):
                                sc = dws_t[:, 9 * m + ti:9 * m + ti + 1]
                                src = kvp[m][:, 1 + dy:1 + SR + dy, 1 + dx:1 + dx + W]
                                d = dst[:, :, :]
                                if ti == 0:
                                    nc.vector.tensor_scalar_mul(d, src, sc)
                                else:
                                    tmp = sp.tile([128, SR, W], BF16, tag=f"dwtmp", name="dwtmp")
                                    nc.vector.tensor_scalar_mul(tmp[:], src, sc)
                                    nc.vector.tensor_add(d, d, tmp[:])
                        for m in range(3 - dw_on_pe, 3):
                            dst = douts[m]
                            for jr in range(0, SR, 4):
                                pdw = pk.tile([128, 512], F32, tag="pkv", name="pdw")
                                for ti, (dy, dx) in enumerate(taps):
                                    lhs = ddg[:, (9 * (m - 1) + ti) * 128:(9 * (m - 1) + ti + 1) * 128]
                                    rhs = kvp[m][:, 1 + jr + dy:1 + jr + dy + 4, 1 + dx:1 + dx + W]
                                    nc.tensor.matmul(pdw[:], lhs, rhs, start=(ti == 0), stop=(ti == 8))
                                nc.vector.tensor_copy(
                                    dst[:, jr:jr + 4, :],
                                    pdw[:].rearrange("p (a b) -> p a b", b=W))

                        # ---- ssq_k ----
                        scr = sp.tile([128, SR, W], BF16, tag="dwtmp", name="scr")
                        nc.scalar.activation(scr[0:64], kvmid[0:64], AF.Square, accum_out=ssqB[:, s:s + 1])

                        # ---- v spill ----
                        nsl = slice(SR * s * W, SR * s * W + SR * W)
                        nc.sync.dma_start(vscr[0:64, nsl], kvmid[64:128].rearrange("p a b -> p (a b)"))
                        nc.sync.dma_start(vscr[64:192, nsl], vB[:].rearrange("p a b -> p (a b)"))

                        # ---- k transpose: ch 128:192 via DMA; ch 0:128 written by PE DW ----
                        kt = sp.tile([128, SR, C], BF16, tag="kt")
                        nc.sync.dma_start_transpose(kt[:, :, 128:192], kvmid[0:64].rearrange("p a b -> p (a b)"))
                        # transposed depthwise for k ch 0:128 on PE:
                        # ktp[px, c] = sum_ch kvp0[ch, px+tap] * ddg0[ch, c]
                        for r0k in range(0, SR, 2):
                            ktp2 = pt.tile([128, 2, 128], F32, tag="ktp", name="ktp2")
                            for jk in range(2):
                                rk = r0k + jk
                                for ti, (dy, dx) in enumerate(taps):
                                    nc.tensor.matmul(ktp2[:, jk, :],
                                                     kvp[0][:, 1 + rk + dy, 1 + dx:1 + dx + 128],
                                                     ddg0[:, ti * 128:(ti + 1) * 128],
                                                     start=(ti == 0), stop=(ti == 8))
                            nc.scalar.copy(kt[:, r0k:r0k + 2, 0:128], ktp2[:])
                        for rk in range(SR):
                            nc.tensor.matmul(gk_hi, kt[:, rk, 0:128], kt[:, rk, 0:128],
                                             start=(s == 0 and rk == 0),
                                             stop=(s == NS - 1 and rk == SR - 1))

                        # ---- q conv (fp8 DR) + attn/gram accumulation ----
                        def attn_mms(qsb, r, kt_, f, l):
                            nc.tensor.matmul(gq_hi, qsb[:, 0:128], qsb[:, 0:128], start=f, stop=l)
                            nc.tensor.matmul(gq_lo, qsb[:, 128:192], qsb[:, 128:192], start=f, stop=l)
                            nc.tensor.matmul(raw01, qsb[:, 0:96], kt_[:, r, 0:96], start=f, stop=l)
                            nc.tensor.matmul(raw23, qsb[:, 96:192], kt_[:, r, 96:192], start=f, stop=l)

                        cur_q = []
                        for r0 in range(0, SR, 2):
                          psq2 = pq.tile([128, 2 * C], F32, tag="psq", name="psq2")
                          qsb2 = qp.tile([128, 2 * C], BF16, tag="qsb", name="qsb2")
                          for r in (r0, r0 + 1):
                            psq = psq2[:, (r - r0) * C:(r - r0) * C + C]
                            if "qconv" in ablate:
                                nc.tensor.matmul(psq, yhi[:, r:r + 2, 0:128],
                                                 wqh[:, :, 0:C],
                                                 start=True, stop=True, perf_mode=DR)
                                qsb = qsb2[:, (r - r0) * C:(r - r0) * C + C]
                                cur_q.append(qsb)
                                if r == r0 + 1:
                                    nc.scalar.copy(qsb2[:], psq2[:])
                                if s > 0:
                                    attn_mms(prev_q[r], r, prev_kt,
                                             f=(s == 1 and r == 0), l=False)
                                continue
                            # hi: 3 row-pair DR MMs (dx=-1,0,+1)
                            for b in range(3):
                                nc.tensor.matmul(psq, yhi[:, r:r + 2, b:b + 128],
                                                 wqh[:, :, b * C:(b + 1) * C],
                                                 start=(b == 0), stop=False, perf_mode=DR)
                            # hi: region-pair (dy=+1: dx=0 & dx=+1)
                            nc.tensor.matmul(psq, yhi[:, (r + 2):(r + 21):18, 1:129],
                                             wqh[:, :, 3 * C:4 * C],
                                             start=False, stop=False, perf_mode=DR)
                            # hi: plain single (dy=+1, dx=-1)
                            nc.tensor.matmul(psq, yhi[:, r + 2, 0:128],
                                             wqh[:, 0, 4 * C:5 * C],
                                             start=False, stop=False)
                            # lo: quad DR (block0), half DR (block1)
                            nc.tensor.matmul(psq, ylo[:, r:r + 2, 0:128],
                                             wql[:, :, 0:C],
                                             start=False, stop=False, perf_mode=DR)
                            nc.tensor.matmul(psq, ylo[:, r:r + 2, 2:130],
                                             wql[:, :, C:2 * C],
                                             start=False, stop=False, perf_mode=DR)
                            # lo: plain (dy=+1 row, 2 taps via dup)
                            nc.tensor.matmul(psq, ylo[:, r + 2, 0:128],
                                             wql[:, 0, 2 * C:3 * C],
                                             start=False, stop=False)
                            # lo: plain 64p (dy=+1, dx=+1)
                            nc.tensor.matmul(psq, ylo[64:128, r + 2, 1:129],
                                             wql[64:128, 0, 3 * C:4 * C],
                                             start=False, stop=True)
                            qsb = qsb2[:, (r - r0) * C:(r - r0) * C + C]
                            cur_q.append(qsb)
                            if r == r0 + 1:
                                nc.scalar.copy(qsb2[:], psq2[:])
                            if s > 0 and "attn" not in ablate:
                                attn_mms(prev_q[r], r, prev_kt,
                                         f=(s == 1 and r == 0), l=False)
                        prev_q, prev_kt = cur_q, kt

                    # drain attn for the last stripe
                    for r in range(SR):
                        if "attn" not in ablate:
                            attn_mms(prev_q[r], r, prev_kt, f=False, l=(r == SR - 1))
                    if "attn" in ablate:
                        nc.tensor.matmul(gq_hi, prev_q[0][:, 0:128], prev_q[0][:, 0:128], start=True, stop=True)
                        nc.tensor.matmul(gq_lo, prev_q[0][:, 128:192], prev_q[0][:, 128:192], start=True, stop=True)
                        nc.tensor.matmul(raw01, prev_q[0][:, 0:96], prev_kt[:, 0, 0:96], start=True, stop=True)
                        nc.tensor.matmul(raw23, prev_q[0][:, 96:192], prev_kt[:, 0, 96:192], start=True, stop=True)

                # ================= phase 2: softmax + MT =================
                with ExitStack() as sctx2:
                    s2 = sctx2.enter_context(tc.tile_pool(name="s2", bufs=1))
                    p2 = sctx2.enter_context(tc.tile_pool(name="p2", bufs=1, space="PSUM"))

                    # ssq_q from gram diagonals
                    scr2 = s2.tile([128, 128], F32)
                    ssqqA = s2.tile([128, 1], F32)
                    nc.vector.scalar_tensor_tensor(scr2[:], gq_hi, 1.0, identF[:],
                                                   AL.mult, AL.mult, accum_out=ssqqA[:])
                    scr2b = s2.tile([64, 64], F32)
                    ssqqB = s2.tile([64, 1], F32)
                    nc.vector.scalar_tensor_tensor(scr2b[:], gq_lo, 1.0, identF[0:64, 0:64],
                                                   AL.mult, AL.mult, accum_out=ssqqB[:])
                    # ssq_k totals: hi from gram diag, lo from per-stripe sums
                    scr3 = s2.tile([128, 128], F32)
                    sskA = s2.tile([128, 1], F32)
                    nc.vector.scalar_tensor_tensor(scr3[:], gk_hi, 1.0, identF[:],
                                                   AL.mult, AL.mult, accum_out=sskA[:])
                    sskB = s2.tile([64, 1], F32)
                    nc.vector.reduce_sum(sskB[:], ssqB[:], axis=mybir.AxisListType.X)

                    def rsqrt(dst, src):
                        nc.vector.reciprocal(dst, src)
                        nc.scalar.activation(dst, dst, AF.Sqrt)

                    rqA = s2.tile([128, 1], F32, name="rqA")
                    rsqrt(rqA[:], ssqqA[:])
                    rqB = s2.tile([64, 1], F32, name="rqB")
                    rsqrt(rqB[:], ssqqB[:])
                    rkA = s2.tile([128, 1], F32, name="rkA")
                    rsqrt(rkA[:], sskA[:])
                    rkB = s2.tile([64, 1], F32, name="rkB")
                    rsqrt(rkB[:], sskB[:])
                    # rq * tau
                    nc.vector.tensor_mul(rqA[:], rqA[:], tauA[:])
                    nc.vector.tensor_mul(rqB[:], rqB[:], tauB[:])

                    # rk rows [1, 96] then broadcast [96, 96] via K=1 matmul with ones
                    nc.sync.dma_start(rscr[0:1, 0:128].rearrange("a b -> b a"), rkA[:])
                    nc.sync.dma_start(rscr[0:1, 128:192].rearrange("a b -> b a"), rkB[:])
                    rkrow = s2.tile([1, 192], F32)
                    nc.sync.dma_start(rkrow[:], rscr)
                    rkrow_b = s2.tile([1, 192], BF16)
                    nc.vector.tensor_copy(rkrow_b[:], rkrow[:])
                    ones1 = s2.tile([1, 96], BF16)
                    nc.vector.memset(ones1[:], 1.0)
                    rkb01p = p2.tile([96, 96], F32)
                    nc.tensor.matmul(rkb01p[:], ones1[:], rkrow_b[0:1, 0:96], start=True, stop=True)
                    rkb23p = p2.tile([96, 96], F32)
                    nc.tensor.matmul(rkb23p[:], ones1[:], rkrow_b[0:1, 96:192], start=True, stop=True)

                    # logits = raw * (rq*tau) * rk
                    l01 = s2.tile([96, 96], F32)
                    nc.scalar.activation(l01[:], raw01, AF.Copy, scale=rqA[0:96, :])
                    nc.vector.tensor_mul(l01[:], l01[:], rkb01p[:])
                    l23 = s2.tile([96, 96], F32)
                    rq23 = s2.tile([96, 1], F32)
                    nc.sync.dma_start(rq23[0:32, :], rqA[96:128, :])
                    nc.sync.dma_start(rq23[32:96, :], rqB[:])
                    nc.scalar.activation(l23[:], raw23, AF.Copy, scale=rq23[:])
                    nc.vector.tensor_mul(l23[:], l23[:], rkb23p[:])

                    # softmax per head-pair with additive block mask -> blockdiag bd (bf16)
                    msk = s2.tile([96, 96], F32)
                    nc.sync.dma_start(msk[:], bmask)
                    bd01 = s2.tile([96, 96], BF16)
                    bd23 = s2.tile([96, 96], BF16)
                    for hb, (lt, bd) in enumerate(((l01, bd01), (l23, bd23))):
                        nc.vector.tensor_add(lt[:], lt[:], msk[:])
                        mx = s2.tile([96, 1], F32, tag=f"mx{hb}", name=f"mx{hb}")
                        nc.vector.reduce_max(mx[:], lt[:], axis=mybir.AxisListType.X)
                        nc.vector.tensor_scalar_mul(mx[:], mx[:], -1.0)
                        ex = s2.tile([96, 96], F32, tag=f"ex{hb}", name=f"ex{hb}")
                        rs = s2.tile([96, 1], F32, tag=f"rs{hb}", name=f"rs{hb}")
                        nc.scalar.activation(ex[:], lt[:], AF.Exp, bias=mx[:], accum_out=rs[:])
                        nc.vector.reciprocal(rs[:], rs[:])
                        nc.vector.tensor_scalar_mul(bd[:], ex[:], rs[:])

                    # MT[d, o] = sum_c attn[c, d] * projT[c, o]
                    mt_hi_p = p2.tile([96, C], F32)
                    nc.tensor.matmul(mt_hi_p[:], bd01[:], prA[:], start=True, stop=True)
                    mt_lo_p = p2.tile([96, C], F32)
                    nc.tensor.matmul(mt_lo_p[:], bd23[:], prB[:], start=True, stop=True)
                    mt_hi = wp.tile([96, C], BF16)
                    nc.scalar.copy(mt_hi[:], mt_hi_p[:])
                    mt_lo = wp.tile([96, C], BF16)
                    nc.scalar.copy(mt_lo[:], mt_lo_p[:])

                # ================= phase 3: out = MT.T @ v, streamed =================
                with ExitStack() as sctx3:
                    s3 = sctx3.enter_context(tc.tile_pool(name="s3", bufs=3))
                    p3 = sctx3.enter_context(tc.tile_pool(name="p3", bufs=2, space="PSUM"))
                    for g in range(NCK // 4):
                        gsl = slice(2048 * g, 2048 * g + 2048)
                        vhi = s3.tile([96, 2048], BF16, tag="vhi")
                        nc.sync.dma_start(vhi[:], vscr[0:96, gsl])
                        vlo = s3.tile([96, 2048], BF16, tag="vlo")
                        nc.sync.dma_start(vlo[:], vscr[96:192, gsl])
                        o1g = s3.tile([128, 2048], BF16, tag="o1")
                        o2g = s3.tile([64, 2048], BF16, tag="o2")
                        for jj in range(4):
                            vsl = slice(512 * jj, 512 * jj + 512)
                            f1 = p3.tile([128, 512], F32, tag="f1")
                            nc.tensor.matmul(f1[:], mt_hi[:, 0:128], vhi[:, vsl], start=True, stop=False)
                            nc.tensor.matmul(f1[:], mt_lo[:, 0:128], vlo[:, vsl], start=False, stop=True)
                            f2 = p3.tile([64, 512], F32, tag="f2")
                            nc.tensor.matmul(f2[:], mt_hi[:, 128:192], vhi[:, vsl], start=True, stop=False)
                            nc.tensor.matmul(f2[:], mt_lo[:, 128:192], vlo[:, vsl], start=False, stop=True)
                            nc.vector.tensor_copy(o1g[:, vsl], f1[:])
                            nc.scalar.copy(o2g[:, vsl], f2[:])
                        nc.scalar.dma_start(out[0:128, gsl], o1g[:])
                        nc.sync.dma_start(out[128:192, gsl], o2g[:])

    nc.compile()
    return nc


# ======================= harness entry point =======================
B = 8
H = 128
_NC = None


def _get_nc():
    global _NC
    if _NC is None:
        _NC = build(H=H)
    return _NC


def _make_in_maps(inputs, H=H):
    import ml_dtypes
    xf = np.ascontiguousarray(inputs["x"], np.float32)
    x = np.asarray(xf, dtype=ml_dtypes.bfloat16)
    x8full = np.asarray(xf, dtype=ml_dtypes.float8_e4m3)
    y = np.asarray(np.ascontiguousarray(inputs["y"], np.float32), dtype=ml_dtypes.float8_e4m3)
    prep = host_prep(inputs["kv_w"], inputs["kv_dw_w"], inputs["q_w"],
                     inputs["q_dw_w"], inputs["proj_w"], inputs["temperature"])
    maps = []
    for b in range(x.shape[0]):
        m = {"x": x[b].reshape(C, H * W), "y": y[b].reshape(C, H * W),
             "x8": x8full[b].reshape(C, H * W)}
        m.update(prep)
        maps.append(m)
    return maps


def _run(inputs, trace=False, trace_kwargs=None):
    from concourse.bass_utils import run_bass_kernel_spmd
    nc = _get_nc()
    res = run_bass_kernel_spmd(nc, _make_in_maps(inputs), core_ids=list(range(B)),
                               trace=trace, trace_kwargs=trace_kwargs or {})
    out = np.stack([np.asarray(res.results[b]["out"]).astype(np.float32).reshape(C, H, W)
                    for b in range(B)])
    return out, res


def kernel(**inputs) -> np.ndarray:
    out, _ = _run(inputs, trace=False)
    return out
